# revision 1
# baseline (speedup 1.0000x reference)
"""Multi-head attention forward on 8 Trainium2 NeuronCores (Bass/Tile).

Problem: nn_MultiHeadAttention — B=8, T=1024, C=768, H=12, D=64, fp32.

Sharding: data-parallel over batch — B=8 -> one batch element per core; weights
broadcast to all cores. No collectives. Host pre-transposes x[b] to x^T [C, T]
and pre-arranges biases; the full output is gathered by stacking per-core
results.

Per-core kernel (all matmul operands float32r — TF32-like fast fp32 PE mode,
full speed at K=128/N>=256, ~1.5e-4 rel err; fp32 matmul proper is 4x slower):
  1. V = x @ Wv (natural [T, C] layout) via matmul(lhsT=xT chunk, rhs=Wv),
     stored into V_aug [128, T/128, H, 65] with a ones column appended per
     head: the ones row of the attention-weighted product later yields the
     softmax denominator for free.
  2. Per head pair p (c_out chunk 128): Q^T chunk via matmul(lhsT=Wq[:,co],
     rhs=xT) -> [128, T] (head-major transposed, exactly what QK^T needs), and
     K^T as TWO zero-padded tiles KTz[hh] [128, T] (the other head's 64
     partitions zeroed). S^T = KTz^T-contract over the FULL K=128 partitions:
     a K=64 matmul runs at half PE rate (419ns vs 202ns measured), padding
     with zeros restores full rate at identical results.
  3. Per head, per j-chunk: S^T[j,:] psum [128, 1024]; P = exp(S^T/8) on
     ScalarE (psum -> sbuf, float32r; no max subtraction needed: logits are
     ~N(0,1), |S|/8 < ~10, exp is ~2ULP-exact); Ytil[65, i*512] accumulates
     matmul(lhsT=V_aug[:, j, h, :], rhs=P chunk) over j.
  4. y^T = Ytil[0:64] * recip(Ytil[64]): DVE copy of the denominator row,
     reciprocal_approx_fast, GpSimd partition_broadcast, DVE multiply into
     Y^T [C, T] (f32r).
  5. out[t, :] = matmul(lhsT=YT[:, k, t128], rhs=Wp[:, k, :]) + bp -> DMA.

Pair p+1's Q^T/K^T projections are emitted MID-pair (between pair p's two
heads) so their PSUM-slot demand lands while attention accumulator slots are
free, letting the projection matmuls fill the ScalarE-bound attention gaps.
PSUM: shared [128,512] pool (bufs=4: QKV accumulators, Ytil accumulators,
projection) + [128,1024] S^T pool (bufs=2) = exactly 8 banks. Input DMAs are
split per k-subtile and issued in consumption order (Wv+xT first) so the
first matmuls start after ~1MB lands; the tiny bias DMAs are issued ahead
of the ~13MB bulk loads so the first bias-add consumers don't queue behind
them. Measured ~195us/core steady-state (slope of an in-kernel repetition
loop vs reps=1), rel err 3.8e-4.
"""
import numpy as np

B, T, C = 8, 1024, 768
H, D = 12, 64
P = 128
KS = C // P          # 6 contraction subtiles
TS = T // P          # 8 t subtiles
NI = T // 512        # 2 i-chunks of 512
N_CORES = 8

_RUNNER_CACHE = {}


def build_nc(reps: int = 1, phases: int = 4, variant: str = "full"):
    import concourse.bacc as bacc
    import concourse.mybir as mybir
    import concourse.tile as tile
    from contextlib import ExitStack

    f32 = mybir.dt.float32
    f32r = mybir.dt.float32r
    AF = mybir.ActivationFunctionType
    ALU = mybir.AluOpType

    nc = bacc.Bacc(num_devices=N_CORES)

    xT_d = nc.dram_tensor("xT", [C, T], f32r, kind="ExternalInput")
    W_d = {w: nc.dram_tensor(f"W{w}", [C, C], f32r, kind="ExternalInput")
           for w in ("q", "k", "v", "p")}
    bqT_d = nc.dram_tensor("bqT", [P, KS], f32, kind="ExternalInput")
    bkT_d = nc.dram_tensor("bkT", [P, KS], f32, kind="ExternalInput")
    bvB_d = nc.dram_tensor("bvB", [P, C], f32, kind="ExternalInput")
    bpB_d = nc.dram_tensor("bpB", [P, C], f32, kind="ExternalInput")
    y_d = nc.dram_tensor("y", [T, C], f32, kind="ExternalOutput")

    with tile.TileContext(nc) as tc, ExitStack() as ctx:
        const = ctx.enter_context(tc.tile_pool(name="const", bufs=1))
        ppool = ctx.enter_context(tc.tile_pool(name="pt", bufs=3))
        npool = ctx.enter_context(tc.tile_pool(name="norm", bufs=4))
        opool = ctx.enter_context(tc.tile_pool(name="out", bufs=2))
        psQ = ctx.enter_context(tc.tile_pool(name="psQ", bufs=4, space="PSUM"))
        psS = ctx.enter_context(tc.tile_pool(name="psS", bufs=2, space="PSUM"))

        def body(_iv=None):
            # ---- loads ----
            xTr = const.tile([P, KS, T], f32r, tag="xT", name="xTr")
            Wr = {}
            for w in ("q", "k", "v"):
                Wr[w] = const.tile([P, KS, C], f32r, tag=f"W{w}", name=f"W{w}r")
            # split loads per k-subtile so the first projection matmuls can
            # start as soon as the first ~1MB lands
            xT_r = xT_d.rearrange("(ks p) t -> p ks t", p=P)
            W_r = {w: W_d[w].rearrange("(ks p) c -> p ks c", p=P) for w in ("q", "k", "v")}
            # tiny bias loads FIRST so they don't queue behind ~13MB of weights
            bqT = const.tile([P, KS], f32, tag="bqT", name="bqT")
            nc.sync.dma_start(bqT[:], bqT_d[:, :])
            bkT = const.tile([P, KS], f32, tag="bkT", name="bkT")
            nc.sync.dma_start(bkT[:], bkT_d[:, :])
            bvB = const.tile([P, C], f32, tag="bvB", name="bvB")
            nc.sync.dma_start(bvB[:], bvB_d[:, :])
            bpB = const.tile([P, C], f32, tag="bpB", name="bpB")
            nc.sync.dma_start(bpB[:], bpB_d[:, :])
            for k in range(KS):
                nc.sync.dma_start(Wr["v"][:, k, :], W_r["v"][:, k, :])
                nc.sync.dma_start(xTr[:, k, :], xT_r[:, k, :])
            for k in range(KS):
                nc.sync.dma_start(Wr["q"][:, k, :], W_r["q"][:, k, :])
            for k in range(KS):
                nc.sync.dma_start(Wr["k"][:, k, :], W_r["k"][:, k, :])
            ones1 = const.tile([P, 1], f32, tag="ones", name="ones1")
            nc.vector.memset(ones1[:], 1.0)
            if phases < 4:
                YTdummy = opool.tile([P, C], f32, tag="ot", name="ytd")
                nc.vector.memset(YTdummy[:], 0.0)
                nc.sync.dma_start(y_d[0:P, :], YTdummy[:])

            # ---- V (natural layout) into V_aug with ones column ----
            V_aug = const.tile([P, TS, H, D + 1], f32r, tag="Vaug", name="Vaug")
            nc.vector.tensor_copy(V_aug[:, :, :, D:D + 1],
                                  ones1[:].to_broadcast([P, TS, H, 1]))
            for ts_ in range(TS):
                psv = [psQ.tile([P, 512], f32, tag="ps512", name="psq") for _ in range(2)]
                for k in range(KS):
                    lhsT = xTr[:, k, ts_ * P:(ts_ + 1) * P]
                    nc.tensor.matmul(psv[0][:], lhsT, Wr["v"][:, k, 0:512],
                                     start=(k == 0), stop=(k == KS - 1))
                    nc.tensor.matmul(psv[1][:, 0:256], lhsT, Wr["v"][:, k, 512:768],
                                     start=(k == 0), stop=(k == KS - 1))
                nc.vector.tensor_tensor(
                    V_aug[:, ts_, 0:8, 0:D],
                    psv[0][:].rearrange("p (h d) -> p h d", h=8),
                    bvB[:, 0:512].rearrange("p (h d) -> p h d", h=8), op=ALU.add)
                nc.vector.tensor_tensor(
                    V_aug[:, ts_, 8:12, 0:D],
                    psv[1][:, 0:256].rearrange("p (h d) -> p h d", h=4),
                    bvB[:, 512:768].rearrange("p (h d) -> p h d", h=4), op=ALU.add)

            if phases < 3:
                return

            # ---- per pair: Q^T/K^T projection (po=p) then attention ----
            # S^T for head h contracts over only 64 dims; a K=64 matmul runs at
            # half rate on the PE (419ns vs 202ns measured). Build KTz tiles
            # with the OTHER head's 64 partitions zeroed and contract over the
            # full 128 partitions: same result, full rate. Interleaving the
            # projections per pair lets them overlap the previous pair's
            # (ScalarE-bound) attention.
            YT = const.tile([P, KS, T], f32r, tag="YTs", name="YT")
            zeros64 = const.tile([64, 512], f32, tag="z64", name="zeros64")
            nc.vector.memset(zeros64[:], 0.0)

            def emit_qkt(p):
                QTp = const.tile([P, T], f32r, tag=f"QT{p % 2}", name="QTp")
                ps = [psQ.tile([P, 512], f32, tag="ps512", name="psq")
                      for _ in range(NI)]
                for k in range(KS):
                    lhsT = Wr["q"][:, k, p * P:(p + 1) * P]
                    for ti in range(NI):
                        nc.tensor.matmul(ps[ti][:], lhsT,
                                         xTr[:, k, ti * 512:(ti + 1) * 512],
                                         start=(k == 0), stop=(k == KS - 1))
                for ti in range(NI):
                    nc.vector.tensor_tensor(
                        QTp[:, ti * 512:(ti + 1) * 512], ps[ti][:],
                        bqT[:, p:p + 1].to_broadcast([P, 512]), op=ALU.add)
                KTz = {}
                ps = [psQ.tile([P, 512], f32, tag="ps512", name="psq")
                      for _ in range(NI)]
                for k in range(KS):
                    lhsT = Wr["k"][:, k, p * P:(p + 1) * P]
                    for ti in range(NI):
                        nc.tensor.matmul(ps[ti][:], lhsT,
                                         xTr[:, k, ti * 512:(ti + 1) * 512],
                                         start=(k == 0), stop=(k == KS - 1))
                for hh in range(2):
                    KTz[hh] = const.tile([P, T], f32r,
                                         tag=f"KTz{hh}_{p % 2}", name="KTz")
                for ti in range(NI):
                    sl = slice(ti * 512, (ti + 1) * 512)
                    nc.vector.tensor_tensor(
                        KTz[0][0:64, sl], ps[ti][0:64, :],
                        bkT[0:64, p:p + 1].to_broadcast([64, 512]), op=ALU.add)
                    nc.scalar.copy(KTz[0][64:128, sl], zeros64[:, :])
                    nc.vector.tensor_tensor(
                        KTz[1][64:128, sl], ps[ti][64:128, :],
                        bkT[64:128, p:p + 1].to_broadcast([64, 512]), op=ALU.add)
                    nc.scalar.copy(KTz[1][0:64, sl], zeros64[:, :])
                return QTp, KTz

            def emit_head(p, hh, QTp, KTz):
                h = 2 * p + hh
                b0 = 64 * hh
                psy = [psQ.tile([P, 512], f32, tag="ps512", name="psy")
                       for _ in range(NI)]
                for j in range(TS):
                    pss = psS.tile([P, 1024], f32, tag="psS", name="pss")
                    for i in range(NI):
                        nc.tensor.matmul(
                            pss[:, i * 512:(i + 1) * 512],
                            KTz[hh][:, j * P:(j + 1) * P],
                            QTp[:, i * 512:(i + 1) * 512],
                            start=True, stop=True)
                    pt = ppool.tile([P, 1024], f32r, tag="pt", name="pt")
                    nc.scalar.activation(pt[:], pss[:], AF.Exp, scale=0.125)
                    for i in range(NI):
                        nc.tensor.matmul(
                            psy[i][0:D + 1, :], V_aug[:, j, h, :],
                            pt[:, i * 512:(i + 1) * 512],
                            start=(j == 0), stop=(j == TS - 1))
                # normalize: y^T = Ytil[0:64] * recip(Ytil[64])
                for i in range(NI):
                    dd = npool.tile([1, 512], f32, tag="dd", name="dd")
                    nc.vector.tensor_copy(dd[0:1, :], psy[i][D:D + 1, :])
                    rr = npool.tile([1, 512], f32, tag="rr", name="rr")
                    nc.vector.reciprocal_approx_fast(rr[0:1, :], dd[0:1, :])
                    rb = npool.tile([D, 512], f32, tag="rb", name="rb")
                    nc.gpsimd.partition_broadcast(rb[:], rr[0:1, :])
                    nc.vector.tensor_tensor(
                        YT[b0:b0 + 64, p, i * 512:(i + 1) * 512],
                        psy[i][0:D, :], rb[:], op=ALU.mult)

            cur = emit_qkt(0)
            for p in range(KS):
                QTp, KTz = cur
                emit_head(p, 0, QTp, KTz)
                if p + 1 < KS:
                    nxt = emit_qkt(p + 1)
                emit_head(p, 1, QTp, KTz)
                if p + 1 < KS:
                    cur = nxt

            if phases < 4:
                return
            # Wp load (deferred; needed only by the output projection)
            Wr["p"] = const.tile([P, KS, C], f32r, tag="Wq", name="Wpr")
            nc.sync.dma_start(Wr["p"][:], W_d["p"].rearrange("(ks p) c -> p ks c", p=P))

            # ---- output projection ----
            for ts_ in range(TS):
                po_ = [psQ.tile([P, 512], f32, tag="ps512", name="psq") for _ in range(2)]
                for k in range(KS):
                    lhsT = YT[:, k, ts_ * P:(ts_ + 1) * P]
                    nc.tensor.matmul(po_[0][:], lhsT, Wr["p"][:, k, 0:512],
                                     start=(k == 0), stop=(k == KS - 1))
                    nc.tensor.matmul(po_[1][:, 0:256], lhsT, Wr["p"][:, k, 512:768],
                                     start=(k == 0), stop=(k == KS - 1))
                ot = opool.tile([P, C], f32, tag="ot", name="ot")
                nc.vector.tensor_tensor(ot[:, 0:512], po_[0][:], bpB[:, 0:512],
                                        op=ALU.add)
                nc.vector.tensor_tensor(ot[:, 512:768], po_[1][:, 0:256],
                                        bpB[:, 512:768], op=ALU.add)
                nc.sync.dma_start(y_d[ts_ * P:(ts_ + 1) * P, :], ot[:])

        if reps == 1:
            body()
        else:
            import concourse.mybir as _mb
            with tc.For_i(0, reps, 1, hint_engines=tuple(_mb.ALL_ENGINES)):
                body()

    nc.compile()
    return nc


class _Runner:
    """Compile once, run many times on the 8 axon-tunneled cores via PJRT."""

    def __init__(self, nc, n_cores):
        import jax
        import concourse.mybir as mybir
        from jax.sharding import Mesh, PartitionSpec
        from jax.experimental.shard_map import shard_map
        from concourse.bass2jax import (
            _bass_exec_p, install_neuronx_cc_hook, partition_id_tensor)

        install_neuronx_cc_hook()
        self.jax = jax
        self.n_cores = n_cores
        partition_name = nc.partition_id_tensor.name if nc.partition_id_tensor else None
        in_names, out_names, out_avals, zero_outs = [], [], [], []
        for alloc in nc.m.functions[0].allocations:
            if not isinstance(alloc, mybir.MemoryLocationSet):
                continue
            name = alloc.memorylocations[0].name
            if alloc.kind == "ExternalInput":
                if name != partition_name:
                    in_names.append(name)
            elif alloc.kind == "ExternalOutput":
                shape = tuple(alloc.tensor_shape)
                dtype = mybir.dt.np(alloc.dtype)
                out_names.append(name)
                out_avals.append(jax.core.ShapedArray(shape, dtype))
                zero_outs.append(np.zeros(shape, dtype))
        self.in_names, self.out_names = in_names, out_names
        self.zero_outs = zero_outs
        all_in = list(in_names) + list(out_names)
        if partition_name is not None:
            all_in.append(partition_name)

        def _body(*args):
            operands = list(args)
            if partition_name is not None:
                operands.append(partition_id_tensor())
            return tuple(_bass_exec_p.bind(
                *operands, out_avals=tuple(out_avals), in_names=tuple(all_in),
                out_names=tuple(out_names), lowering_input_output_aliases=(),
                sim_require_finite=True, sim_require_nnan=True, nc=nc))

        devices = jax.devices()[:n_cores]
        self.mesh = Mesh(np.asarray(devices), ("core",))
        spec = PartitionSpec("core")
        self.fn = jax.jit(
            shard_map(_body, mesh=self.mesh,
                      in_specs=(spec,) * (len(in_names) + len(out_names)),
                      out_specs=(spec,) * len(out_names), check_rep=False),
            keep_unused=True)

    def stage(self, in_maps):
        import jax
        from jax.sharding import PartitionSpec
        concat = [
            np.concatenate([np.asarray(in_maps[c][n]) for c in range(self.n_cores)], axis=0)
            for n in self.in_names
        ] + [np.concatenate([z] * self.n_cores, axis=0) for z in self.zero_outs]
        sharding = jax.sharding.NamedSharding(self.mesh, PartitionSpec("core"))
        return [jax.device_put(a, sharding) for a in concat]

    def run(self, staged):
        outs = self.fn(*staged)
        self.jax.block_until_ready(outs)
        return outs

    def run_to_maps(self, staged):
        outs = self.run(staged)
        res = []
        for c in range(self.n_cores):
            m = {}
            for i, n in enumerate(self.out_names):
                g = np.asarray(outs[i])
                per = g.shape[0] // self.n_cores
                m[n] = g[c * per:(c + 1) * per]
            res.append(m)
        return res


def get_runner(reps: int = 1, phases: int = 4, variant: str = "full"):
    key = (reps, phases, variant)
    if key not in _RUNNER_CACHE:
        nc = build_nc(reps, phases, variant)
        _RUNNER_CACHE[key] = _Runner(nc, N_CORES)
    return _RUNNER_CACHE[key]


def make_in_maps(x, Wq, bq, Wk, bk, Wv, bv, Wp, bp):
    x = np.asarray(x, dtype=np.float32)
    weights = {
        "Wq": np.asarray(Wq, np.float32), "Wk": np.asarray(Wk, np.float32),
        "Wv": np.asarray(Wv, np.float32), "Wp": np.asarray(Wp, np.float32),
    }
    bqT = np.ascontiguousarray(np.asarray(bq, np.float32).reshape(KS, P).T)
    bkT = np.ascontiguousarray(np.asarray(bk, np.float32).reshape(KS, P).T)
    bvB = np.ascontiguousarray(np.broadcast_to(np.asarray(bv, np.float32), (P, C)))
    bpB = np.ascontiguousarray(np.broadcast_to(np.asarray(bp, np.float32), (P, C)))
    in_maps = []
    for b in range(B):
        in_maps.append({
            "xT": np.ascontiguousarray(x[b].T),
            "Wq": weights["Wq"], "Wk": weights["Wk"],
            "Wv": weights["Wv"], "Wp": weights["Wp"],
            "bqT": bqT, "bkT": bkT, "bvB": bvB, "bpB": bpB,
        })
    return in_maps


def kernel(x, Wq, bq, Wk, bk, Wv, bv, Wp, bp):
    runner = get_runner(reps=1)
    in_maps = make_in_maps(x, Wq, bq, Wk, bk, Wv, bv, Wp, bp)
    staged = runner.stage(in_maps)
    res = runner.run_to_maps(staged)
    return np.stack([res[b]["y"] for b in range(B)], axis=0)



# revision 22
# speedup vs baseline: 1.1015x; 1.1015x over previous
"""Multi-head attention forward on 8 Trainium2 NeuronCores (Bass/Tile).

Problem: nn_MultiHeadAttention — B=8, T=1024, C=768, H=12, D=64, fp32 in/out.

Sharding: data-parallel over batch — one batch element per core; weights
broadcast. No collectives. Host pre-transposes x[b] to x^T [C, T], converts
to bf16, and upcasts the bf16 output back to fp32.

Design (vs the f32r baseline at ~225us single-shot, measured ~223us):
  - bf16 data path (PSUM fp32): halves HBM bytes and DVE element time and
    enables fast weight load; rel err ~6.5e-3 (tolerance 2e-2).
  - S^T per head is a K=64 contraction: the two heads of a pair run as
    row-tiled CONCURRENT matmuls (head0 at partitions 0-63 -> row groups
    0-1, head1 at 64-127; tile_position auto-derives from base_partition),
    so no zero-padded K=128 tiles and no 2x MAC waste.
  - Exp runs as TWO [128,1024] ACTs per (pair, key-chunk j), one per query
    half on its own 2-bank PSUM tile (tags sA/sB): S(j+1)'s first matmuls
    only wait the first half's ACT, which shortens the serial S->exp chain
    while ScalarE stays ~saturated (96 ACTs x 1.15us = 110us).
  - PV is deferred one pair (pt kept in SBUF [P,TS,2,2,512]) and consumed
    during the next pair's S/exp phase: [65,1024] accumulators with the
    ones-column denominator trick; one normalize chain per (pair, head).
    PV pieces sit at slots 1-3/5-7 so the psV write-after-read handoff on
    the previous head's normalize never head-of-line-blocks the S matmuls
    behind them in the TensorE FIFO.
  - V projection runs inside pair0's slots (per-chunk dependency); QK
    projections are emitted as per-k units, 3 per slot; the output
    projection is spread across pairs as per-k half-chunks accumulated in
    SBUF fp32, with k=4,5 in a two-wave tail on 8 borrowed PSUM buffers.
  - DMA: few big issues ordered xT/Wq/Wk first; Wv and Wp transfers are
    GATED behind Q/K projection completion via a tile-corner copy (DMA
    transfers share bandwidth, so early Wv/Wp starts would starve the
    critical-path Wk transfer).
  - A 4-link PE warmup chain runs during the DMA prologue so the HAM clock
    gate is released (2.4 GHz) before real matmuls start.
  - PSUM: S 2+2 banks + V/PV 2 + projection 1 + output 1 = 8.
  - Known HW quirk: custom-DVE reciprocal_approx_fast reads garbage from
    PSUM on hardware (sim is fine) -> denominator row is staged to SBUF.
"""
import numpy as np

B, T, C = 8, 1024, 768
H, D = 12, 64
P = 128
KS = C // P          # 6 contraction subtiles
TS = 8               # 8 t subtiles
NP = H // 2          # 6 head pairs
NI = 2
N_CORES = 8

_RUNNER_CACHE = {}


def build_nc(reps: int = 1, phases: int = 4, variant: str = "full"):
    import concourse.bacc as bacc
    import concourse.mybir as mybir
    import concourse.tile as tile
    from contextlib import ExitStack

    f32 = mybir.dt.float32
    bf16 = mybir.dt.bfloat16
    AF = mybir.ActivationFunctionType
    ALU = mybir.AluOpType

    nc = bacc.Bacc(num_devices=N_CORES)

    xT_d = nc.dram_tensor("xT", [C, T], bf16, kind="ExternalInput")
    W_d = {w: nc.dram_tensor(f"W{w}", [C, C], bf16, kind="ExternalInput")
           for w in ("q", "k", "v", "p")}
    bqT_d = nc.dram_tensor("bqT", [P, KS], f32, kind="ExternalInput")
    bkT_d = nc.dram_tensor("bkT", [P, KS], f32, kind="ExternalInput")
    bvB_d = nc.dram_tensor("bvB", [P, C], f32, kind="ExternalInput")
    bpB_d = nc.dram_tensor("bpB", [P, C], f32, kind="ExternalInput")
    y_d = nc.dram_tensor("y", [T, C], bf16, kind="ExternalOutput")

    with tile.TileContext(nc) as tc, ExitStack() as ctx:
        const = ctx.enter_context(tc.tile_pool(name="const", bufs=1))
        ptp = ctx.enter_context(tc.tile_pool(name="ptp", bufs=1))
        npool = ctx.enter_context(tc.tile_pool(name="norm", bufs=2))
        opool = ctx.enter_context(tc.tile_pool(name="out", bufs=2))
        psS = ctx.enter_context(tc.tile_pool(name="psS", bufs=1, space="PSUM"))
        psV = ctx.enter_context(tc.tile_pool(name="psV", bufs=1, space="PSUM"))
        psP1 = ctx.enter_context(tc.tile_pool(name="psP1", bufs=1, space="PSUM"))
        psP2 = ctx.enter_context(tc.tile_pool(name="psP2", bufs=1, space="PSUM"))

        def body(_iv=None):
            # ---- warmup scratch + chain source ----
            wsrc = const.tile([P, 640], bf16, tag="wsrc", name="wsrc")
            nc.vector.memset(wsrc[:], 1.0)

            # ---- loads: few, ordered issues (each issue ~0.6us serial) ----
            xTr = const.tile([P, KS, T], bf16, tag="xT", name="xTr")
            Wr = {w: const.tile([P, KS, C], bf16, tag=f"W{w}", name=f"W{w}r")
                  for w in ("q", "k", "v", "p")}
            xT_r = xT_d.rearrange("(ks p) t -> p ks t", p=P)
            W_r = {w: W_d[w].rearrange("(ks p) c -> p ks c", p=P)
                   for w in ("q", "k", "v", "p")}
            nc.sync.dma_start(xTr[:, 0, :], xT_r[:, 0, :])
            nc.sync.dma_start(Wr["q"][:, 0, :], W_r["q"][:, 0, :])
            nc.sync.dma_start(xTr[:, 1:KS, :], xT_r[:, 1:KS, :])
            nc.sync.dma_start(Wr["q"][:, 1:KS, :], W_r["q"][:, 1:KS, :])
            nc.sync.dma_start(Wr["k"][:], W_r["k"][:])
            bqT = const.tile([P, KS], f32, tag="bqT", name="bqT")
            nc.sync.dma_start(bqT[:], bqT_d[:, :])
            bkT = const.tile([P, KS], f32, tag="bkT", name="bkT")
            nc.sync.dma_start(bkT[:], bkT_d[:, :])
            bvB = const.tile([P, C], f32, tag="bvB", name="bvB")
            nc.sync.dma_start(bvB[:], bvB_d[:, :])
            bpB = const.tile([P, C], f32, tag="bpB", name="bpB")
            nc.sync.dma_start(bpB[:], bpB_d[:, :])
            # Wv / Wp DMAs are gated: DMA transfers share bandwidth, so an
            # early Wv/Wp start would starve the critical-path Wk transfer.
            # A 2-element copy from QT/KT into the tile corner makes the DMA
            # wait (WAR) until the Q / K projections are done.

            # ---- PE warmup chain: keep HAM un-throttled during DMA wait.
            # Each link: 2 MMs then a DVE copy that the next link's rhs
            # region depends on, pacing ~1us/link without blocking DMAs.
            for wl in range(4):
                psw = psV.tile([P, 1024], f32, tag="psv", name="psw")
                nc.tensor.matmul(psw[:, 0:512], wsrc[:, 0:P],
                                 wsrc[:, P:P + 512], start=True, stop=True)
                nc.tensor.matmul(psw[:, 512:1024], wsrc[:, 0:P],
                                 wsrc[:, P:P + 512], start=True, stop=True)
                nc.vector.tensor_copy(wsrc[:, 128 + wl:129 + wl],
                                      psw[:, wl:wl + 1])

            ones1 = const.tile([P, 1], f32, tag="ones", name="ones1")
            nc.vector.memset(ones1[:], 1.0)
            V_aug = const.tile([P, TS, H, D + 1], bf16, tag="Vaug", name="Vaug")
            nc.vector.tensor_copy(V_aug[:, :, :, D:D + 1],
                                  ones1[:].to_broadcast([P, TS, H, 1]))

            YT = const.tile([P, KS, T], bf16, tag="YTs", name="YT")
            YACC = const.tile([P, TS, C], f32, tag="YACC", name="YACC")

            # ---------------- emission helpers ----------------
            qk_state = {}

            def emit_qk_unit(p, w, ti, k):
                """One k-subtile of the QT/KT(p) ti-half projection. The
                [128,512] accumulator (psP1, single buffer) is allocated at
                k==0 and drained+biased at k==KS-1, so the four passes of a
                pair rotate through one bank without blocking out-chunks."""
                dst, bias = (QT, bqT) if w == "q" else (KT, bkT)
                if k == 0:
                    qk_state[(w, ti)] = psP1.tile([P, 512], f32, tag="psp1",
                                                  name="psp1")
                ps = qk_state[(w, ti)]
                nc.tensor.matmul(ps[:], Wr[w][:, k, p * P:(p + 1) * P],
                                 xTr[:, k, ti * 512:(ti + 1) * 512],
                                 start=(k == 0), stop=(k == KS - 1))
                if k == KS - 1:
                    nc.vector.tensor_tensor(
                        dst[p % 2][:, ti * 512:(ti + 1) * 512], ps[:],
                        bias[:, p:p + 1].to_broadcast([P, 512]), op=ALU.add)

            QK_UNITS = [(w, ti, k) for w, ti in
                        (("q", 0), ("q", 1), ("k", 0), ("k", 1))
                        for k in range(KS)]

            def emit_qk(p, w):
                for ti in range(NI):
                    for k in range(KS):
                        emit_qk_unit(p, w, ti, k)

            def emit_v(ts_):
                """V projection t-chunk into V_aug, one [128,1024] tile,
                one merged bias-add drain."""
                ps = psV.tile([P, 1024], f32, tag="psv", name="psva")
                for k in range(KS):
                    lhsT = xTr[:, k, ts_ * P:(ts_ + 1) * P]
                    nc.tensor.matmul(ps[:, 0:512], lhsT, Wr["v"][:, k, 0:512],
                                     start=(k == 0), stop=(k == KS - 1))
                    nc.tensor.matmul(ps[:, 512:768], lhsT, Wr["v"][:, k, 512:768],
                                     start=(k == 0), stop=(k == KS - 1))
                nc.vector.tensor_tensor(
                    V_aug[:, ts_, :, 0:D],
                    ps[:, 0:768].rearrange("p (h d) -> p h d", h=H),
                    bvB[:, 0:768].rearrange("p (h d) -> p h d", h=H),
                    op=ALU.add)

            def emit_s_exp(p, j, ptb):
                """S^T both heads for key-chunk j + one exp ACT.
                Per head: 1 LDW + 2 pipelined MMs; heads run concurrently
                on disjoint row groups (base partition 0 / 64)."""
                pst = {0: psS.tile([P, 1024], f32, tag="sA", name="sA"),
                       1: psS.tile([P, 1024], f32, tag="sB", name="sB")}
                for i in range(NI):
                    for hh in range(2):
                        b0 = 64 * hh
                        nc.tensor.matmul(
                            pst[i][:, hh * 512:(hh + 1) * 512],
                            KT[p % 2][b0:b0 + 64, j * P:(j + 1) * P],
                            QT[p % 2][b0:b0 + 64, i * 512:(i + 1) * 512],
                            start=True, stop=True)
                    # exp of query-half i for both heads; frees tile i for
                    # S(j+1) while the other half's ACT still runs.
                    nc.scalar.activation(ptb[:, j, i, :, :], pst[i][:], AF.Exp,
                                         scale=0.125)

            def emit_pv_j(p, hh, j, ptb, ps):
                """One key-chunk of the PV accumulation for head 2p+hh."""
                h = 2 * p + hh
                lhsT = V_aug[:, j, h, :]
                for i in range(NI):
                    nc.tensor.matmul(
                        ps[0:D + 1, i * 512:(i + 1) * 512], lhsT,
                        ptb[:, j, i, hh, :],
                        start=(j == 0), stop=(j == TS - 1))

            def emit_pv_norm(p, hh, ps, stage_scalar=False):
                """Normalize by the ones-row denominator into YT."""
                b0 = 64 * hh
                # stage the denominator row to SBUF first: the custom-DVE
                # reciprocal reads garbage from PSUM on real HW (sim is fine).
                # In the tail ScalarE is idle, so stage there instead of DVE.
                dd = npool.tile([1, 1024], f32, tag="dd", name="dd")
                if stage_scalar:
                    nc.scalar.copy(dd[0:1, :], ps[D:D + 1, :])
                else:
                    nc.vector.tensor_copy(dd[0:1, :], ps[D:D + 1, :])
                rr = npool.tile([1, 1024], f32, tag="rr", name="rr")
                nc.vector.reciprocal_approx_fast(rr[0:1, :], dd[0:1, :])
                rb = npool.tile([D, 1024], f32, tag="rb", name="rb")
                nc.gpsimd.partition_broadcast(rb[:], rr[0:1, :])
                nc.vector.tensor_tensor(YT[b0:b0 + 64, p, :], ps[0:D, :],
                                        rb[:], op=ALU.mult)

            otile = {}

            def emit_out_half(ks_, ts_, half, pool=None, ps=None):
                """Half an output-projection chunk: one MM (cols 0-511 or
                512-767) over contraction subtiles ks_, one DVE add."""
                c0, c1 = (0, 512) if half == 0 else (512, 768)
                w_ = c1 - c0
                if ps is None:
                    pool = pool if pool is not None else psP2
                    tg = "psp1" if pool is psP1 else "pso"
                    ps = pool.tile([P, 512], f32, tag=tg, name="pso")
                for n, k in enumerate(ks_):
                    nc.tensor.matmul(ps[:, 0:w_],
                                     YT[:, k, ts_ * P:(ts_ + 1) * P],
                                     Wr["p"][:, k, c0:c1],
                                     start=(n == 0), stop=(n == len(ks_) - 1))
                first, last = (ks_[0] == 0), (ks_[-1] == KS - 1)
                if first:
                    nc.vector.tensor_tensor(YACC[:, ts_, c0:c1], ps[:, 0:w_],
                                            bpB[:, c0:c1], op=ALU.add)
                elif not last:
                    nc.vector.tensor_tensor(YACC[:, ts_, c0:c1],
                                            YACC[:, ts_, c0:c1], ps[:, 0:w_],
                                            op=ALU.add)
                else:
                    if half == 0:
                        otile[ts_] = opool.tile([P, C], bf16, tag="ot",
                                                name="ot")
                    ot = otile[ts_]
                    nc.vector.tensor_tensor(ot[:, c0:c1], YACC[:, ts_, c0:c1],
                                            ps[:, 0:w_], op=ALU.add)
                    if half == 1:
                        nc.sync.dma_start(y_d[ts_ * P:(ts_ + 1) * P, :], ot[:])

            # ---------------- prologue: QT/KT for pair 0 ----------------
            QT = {}
            KT = {}
            for pp in range(2):
                QT[pp] = const.tile([P, T], bf16, tag=f"QT{pp}", name="QT")
                KT[pp] = const.tile([P, T], bf16, tag=f"KT{pp}", name="KT")
            emit_qk(0, "q")
            emit_qk(0, "k")
            # Release the gated Wv / Wp loads now that Q/K are projected:
            # the corner copies below read QT/KT, so the DMAs (WAR on the
            # tiles) cannot start before the Q / K projections finish.
            nc.vector.tensor_copy(Wr["v"][0:1, 0, 0:2], QT[0][0:1, 0:2])
            nc.sync.dma_start(Wr["v"][:], W_r["v"][:])
            nc.vector.tensor_copy(Wr["p"][0:1, 0, 0:2], KT[0][0:1, 0:2])
            nc.sync.dma_start(Wr["p"][:], W_r["p"][:])

            # ---------------- pair loop ----------------
            ptbs = {pp: ptp.tile([P, TS, NI, 2, 512], bf16, tag=f"pt{pp}", name="pt")
                    for pp in range(2)}
            for p in range(NP):
                ptb = ptbs[p % 2]
                ptb_prev = ptbs[(p - 1) % 2]
                pv_tile = None
                for j in range(TS):
                    emit_s_exp(p, j, ptb)
                    if p == 0:
                        emit_v(j)
                        for u in range(3 * j, 3 * j + 3):
                            w, ti, k = QK_UNITS[u]
                            emit_qk_unit(1, w, ti, k)
                    else:
                        # PV(p-1): head0 at slots 1-3, head1 at 5-7 (3/3/2
                        # key-chunks). Slots 0 and 4 are handoff slack: the
                        # psV WAR on the previous head's normalize chain
                        # resolves before the next head's first piece, so
                        # stalled PV matmuls never head-of-line-block the S
                        # matmuls behind them in the TensorE FIFO.
                        PVJ = {1: (0, 1, 2), 2: (3, 4, 5), 3: (6, 7),
                               5: (0, 1, 2), 6: (3, 4, 5), 7: (6, 7)}
                        if j in PVJ:
                            hh = 0 if j < 4 else 1
                            if j in (1, 5):
                                pv_tile = psV.tile([P, 1024], f32, tag="psv",
                                                   name="psy")
                            for jj in PVJ[j]:
                                emit_pv_j(p - 1, hh, jj, ptb_prev, pv_tile)
                            if j in (3, 7):
                                emit_pv_norm(p - 1, hh, pv_tile)
                        if p < NP - 1:
                            for u in range(3 * j, 3 * j + 3):
                                w, ti, k = QK_UNITS[u]
                                emit_qk_unit(p + 1, w, ti, k)
                        if p >= 2:
                            emit_out_half((p - 2,), j, 0)
                            emit_out_half((p - 2,), j, 1)

            # ---------------- tail ----------------
            # The two PV(p5) passes run CONCURRENTLY: head1 borrows a psS
            # buffer (free once the last ACT drains), so its accumulation
            # does not wait for head0's normalize.
            pl = ptbs[(NP - 1) % 2]
            tail_ps = {}
            t0 = psV.tile([P, 1024], f32, tag="psv", name="psy")
            t1 = psS.tile([P, 1024], f32, tag="sA", name="psyT")
            tx = psS.tile([P, 1024], f32, tag="sB", name="psoX")
            for j in range(TS):
                emit_pv_j(NP - 1, 0, j, pl, t0)
                emit_pv_j(NP - 1, 1, j, pl, t1)
            emit_pv_norm(NP - 1, 0, t0, stage_scalar=True)
            emit_pv_norm(NP - 1, 1, t1, stage_scalar=True)
            # Two waves of 4 t-chunks on 8 half-buffers. Within a wave all
            # k4 matmuls are emitted first (they only need YT[:,4], ready at
            # tail start, so they fill the PE while the normalize chains
            # run); the k5 matmuls + drains follow.
            def tail_bufs(ts_, half):
                idx = (ts_ % 4) * 2 + half
                return [tx[:, 0:512], tx[:, 512:1024],
                        t0[:, 0:512], t0[:, 512:1024],
                        t1[:, 0:512], t1[:, 512:1024],
                        None, None][idx]

            for wave in range(2):
                tss = range(wave * 4, wave * 4 + 4)
                for ts_ in tss:
                    for half in range(2):
                        ps = tail_bufs(ts_, half)
                        c0, c1 = (0, 512) if half == 0 else (512, 768)
                        if ps is None:
                            pool = psP1 if half == 0 else psP2
                            ps = pool.tile([P, 512], f32,
                                           tag="psp1" if half == 0 else "pso",
                                           name="pso")
                        tail_ps[(ts_, half)] = ps
                        nc.tensor.matmul(ps[:, 0:c1 - c0],
                                         YT[:, KS - 2, ts_ * P:(ts_ + 1) * P],
                                         Wr["p"][:, KS - 2, c0:c1],
                                         start=True, stop=False)
                for ts_ in tss:
                    for half in range(2):
                        ps = tail_ps[(ts_, half)]
                        c0, c1 = (0, 512) if half == 0 else (512, 768)
                        nc.tensor.matmul(ps[:, 0:c1 - c0],
                                         YT[:, KS - 1, ts_ * P:(ts_ + 1) * P],
                                         Wr["p"][:, KS - 1, c0:c1],
                                         start=False, stop=True)
                        if half == 0:
                            otile[ts_] = opool.tile([P, C], bf16, tag="ot",
                                                    name="ot")
                        ot = otile[ts_]
                        nc.vector.tensor_tensor(ot[:, c0:c1],
                                                YACC[:, ts_, c0:c1],
                                                ps[:, 0:c1 - c0], op=ALU.add)
                        if half == 1:
                            nc.sync.dma_start(y_d[ts_ * P:(ts_ + 1) * P, :],
                                              ot[:])

        if reps == 1:
            body()
        else:
            import concourse.mybir as _mb
            with tc.For_i(0, reps, 1, hint_engines=tuple(_mb.ALL_ENGINES)):
                body()

    nc.compile()
    return nc


class _Runner:
    """Compile once, run many times on the 8 axon-tunneled cores via PJRT."""

    def __init__(self, nc, n_cores):
        import jax
        import concourse.mybir as mybir
        from jax.sharding import Mesh, PartitionSpec
        from jax.experimental.shard_map import shard_map
        from concourse.bass2jax import (
            _bass_exec_p, install_neuronx_cc_hook, partition_id_tensor)

        install_neuronx_cc_hook()
        self.jax = jax
        self.n_cores = n_cores
        partition_name = nc.partition_id_tensor.name if nc.partition_id_tensor else None
        in_names, out_names, out_avals, zero_outs = [], [], [], []
        for alloc in nc.m.functions[0].allocations:
            if not isinstance(alloc, mybir.MemoryLocationSet):
                continue
            name = alloc.memorylocations[0].name
            if alloc.kind == "ExternalInput":
                if name != partition_name:
                    in_names.append(name)
            elif alloc.kind == "ExternalOutput":
                shape = tuple(alloc.tensor_shape)
                dtype = mybir.dt.np(alloc.dtype)
                out_names.append(name)
                out_avals.append(jax.core.ShapedArray(shape, dtype))
                zero_outs.append(np.zeros(shape, dtype))
        self.in_names, self.out_names = in_names, out_names
        self.zero_outs = zero_outs
        all_in = list(in_names) + list(out_names)
        if partition_name is not None:
            all_in.append(partition_name)

        def _body(*args):
            operands = list(args)
            if partition_name is not None:
                operands.append(partition_id_tensor())
            return tuple(_bass_exec_p.bind(
                *operands, out_avals=tuple(out_avals), in_names=tuple(all_in),
                out_names=tuple(out_names), lowering_input_output_aliases=(),
                sim_require_finite=True, sim_require_nnan=True, nc=nc))

        devices = jax.devices()[:n_cores]
        self.mesh = Mesh(np.asarray(devices), ("core",))
        spec = PartitionSpec("core")
        self.fn = jax.jit(
            shard_map(_body, mesh=self.mesh,
                      in_specs=(spec,) * (len(in_names) + len(out_names)),
                      out_specs=(spec,) * len(out_names), check_rep=False),
            keep_unused=True)

    def stage(self, in_maps):
        import jax
        from jax.sharding import PartitionSpec
        concat = [
            np.concatenate([np.asarray(in_maps[c][n]) for c in range(self.n_cores)], axis=0)
            for n in self.in_names
        ] + [np.concatenate([z] * self.n_cores, axis=0) for z in self.zero_outs]
        sharding = jax.sharding.NamedSharding(self.mesh, PartitionSpec("core"))
        return [jax.device_put(a, sharding) for a in concat]

    def run(self, staged):
        outs = self.fn(*staged)
        self.jax.block_until_ready(outs)
        return outs

    def run_to_maps(self, staged):
        outs = self.run(staged)
        res = []
        for c in range(self.n_cores):
            m = {}
            for i, n in enumerate(self.out_names):
                g = np.asarray(outs[i])
                per = g.shape[0] // self.n_cores
                m[n] = g[c * per:(c + 1) * per]
            res.append(m)
        return res


def get_runner(reps: int = 1, phases: int = 4, variant: str = "full"):
    key = (reps, phases, variant)
    if key not in _RUNNER_CACHE:
        nc = build_nc(reps, phases, variant)
        _RUNNER_CACHE[key] = _Runner(nc, N_CORES)
    return _RUNNER_CACHE[key]


def make_in_maps(x, Wq, bq, Wk, bk, Wv, bv, Wp, bp):
    import ml_dtypes
    bf = ml_dtypes.bfloat16
    x = np.asarray(x, dtype=np.float32)
    weights = {
        "Wq": np.asarray(Wq, bf), "Wk": np.asarray(Wk, bf),
        "Wv": np.asarray(Wv, bf), "Wp": np.asarray(Wp, bf),
    }
    bqT = np.ascontiguousarray(np.asarray(bq, np.float32).reshape(KS, P).T)
    bkT = np.ascontiguousarray(np.asarray(bk, np.float32).reshape(KS, P).T)
    bvB = np.ascontiguousarray(np.broadcast_to(np.asarray(bv, np.float32), (P, C)))
    bpB = np.ascontiguousarray(np.broadcast_to(np.asarray(bp, np.float32), (P, C)))
    in_maps = []
    for b in range(B):
        in_maps.append({
            "xT": np.ascontiguousarray(x[b].T.astype(bf)),
            "Wq": weights["Wq"], "Wk": weights["Wk"],
            "Wv": weights["Wv"], "Wp": weights["Wp"],
            "bqT": bqT, "bkT": bkT, "bvB": bvB, "bpB": bpB,
        })
    return in_maps


def kernel(x, Wq, bq, Wk, bk, Wv, bv, Wp, bp):
    runner = get_runner(reps=1)
    in_maps = make_in_maps(x, Wq, bq, Wk, bk, Wv, bv, Wp, bp)
    staged = runner.stage(in_maps)
    res = runner.run_to_maps(staged)
    return np.stack([np.asarray(res[b]["y"], np.float32) for b in range(B)],
                    axis=0)


# revision 24
# speedup vs baseline: 1.2348x; 1.1210x over previous
"""Multi-head attention forward on 8 Trainium2 NeuronCores (Bass/Tile).

Problem: nn_MultiHeadAttention — B=8, T=1024, C=768, H=12, D=64, fp32 in/out.

Sharding: data-parallel over batch — one batch element per core; weights
broadcast. No collectives. Host pre-transposes x[b] to x^T [C, T], converts
to bf16, and upcasts the bf16 output back to fp32.

Design (vs the f32r baseline at ~225us single-shot, measured ~223us):
  - bf16 data path (PSUM fp32): halves HBM bytes and DVE element time and
    enables fast weight load; rel err ~6.5e-3 (tolerance 2e-2).
  - S^T per head is a K=64 contraction: the two heads of a pair run as
    row-tiled CONCURRENT matmuls (head0 at partitions 0-63 -> row groups
    0-1, head1 at 64-127; tile_position auto-derives from base_partition),
    so no zero-padded K=128 tiles and no 2x MAC waste.
  - Exp runs as TWO [128,1024] ACTs per (pair, key-chunk j), one per query
    half on its own 2-bank PSUM tile (tags sA/sB): S(j+1)'s first matmuls
    only wait the first half's ACT, which shortens the serial S->exp chain
    while ScalarE stays ~saturated (96 ACTs x 1.15us = 110us).
  - PV is deferred one pair (pt kept in SBUF [P,TS,2,2,512]) and consumed
    during the next pair's S/exp phase: [65,1024] accumulators with the
    ones-column denominator trick; one normalize chain per (pair, head).
    PV pieces sit at slots 1-3/5-7 so the psV write-after-read handoff on
    the previous head's normalize never head-of-line-blocks the S matmuls
    behind them in the TensorE FIFO.
  - V projection runs inside pair0's slots (per-chunk dependency); QK
    projections are emitted as per-k units, 3 per slot; the output
    projection is spread across pairs as per-k half-chunks accumulated in
    SBUF fp32, with k=4,5 in a two-wave tail on 8 borrowed PSUM buffers.
  - DMA: few big issues ordered xT/Wq/Wk first; Wv and Wp transfers are
    GATED behind Q/K projection completion via a tile-corner copy (DMA
    transfers share bandwidth, so early Wv/Wp starts would starve the
    critical-path Wk transfer).
  - A 4-link PE warmup chain runs during the DMA prologue so the HAM clock
    gate is released (2.4 GHz) before real matmuls start.
  - PSUM: S 2+2 banks + V/PV 2 + projection 1 + output 1 = 8.
  - Known HW quirk: custom-DVE reciprocal_approx_fast reads garbage from
    PSUM on hardware (sim is fine) -> denominator row is staged to SBUF.
"""
import numpy as np

B, T, C = 8, 1024, 768
H, D = 12, 64
P = 128
KS = C // P          # 6 contraction subtiles
TS = 8               # 8 t subtiles
NP = H // 2          # 6 head pairs
NI = 2
N_CORES = 8

_RUNNER_CACHE = {}


def build_nc(reps: int = 1, phases: int = 4, variant: str = "full"):
    import concourse.bacc as bacc
    import concourse.mybir as mybir
    import concourse.tile as tile
    from contextlib import ExitStack

    f32 = mybir.dt.float32
    bf16 = mybir.dt.bfloat16
    AF = mybir.ActivationFunctionType
    ALU = mybir.AluOpType

    nc = bacc.Bacc(num_devices=N_CORES)

    xT_d = nc.dram_tensor("xT", [C, T], bf16, kind="ExternalInput")
    W_d = {w: nc.dram_tensor(f"W{w}", [C, C], bf16, kind="ExternalInput")
           for w in ("q", "k", "v", "p")}
    bqT_d = nc.dram_tensor("bqT", [P, KS], f32, kind="ExternalInput")
    bkT_d = nc.dram_tensor("bkT", [P, KS], f32, kind="ExternalInput")
    bvB_d = nc.dram_tensor("bvB", [P, C], f32, kind="ExternalInput")
    bpB_d = nc.dram_tensor("bpB", [P, C], f32, kind="ExternalInput")
    y_d = nc.dram_tensor("y", [T, C], bf16, kind="ExternalOutput")

    with tile.TileContext(nc) as tc, ExitStack() as ctx:
        const = ctx.enter_context(tc.tile_pool(name="const", bufs=1))
        ptp = ctx.enter_context(tc.tile_pool(name="ptp", bufs=1))
        npool = ctx.enter_context(tc.tile_pool(name="norm", bufs=2))
        opool = ctx.enter_context(tc.tile_pool(name="out", bufs=2))
        psS = ctx.enter_context(tc.tile_pool(name="psS", bufs=1, space="PSUM"))
        psV = ctx.enter_context(tc.tile_pool(name="psV", bufs=1, space="PSUM"))
        psP1 = ctx.enter_context(tc.tile_pool(name="psP1", bufs=1, space="PSUM"))
        psP2 = ctx.enter_context(tc.tile_pool(name="psP2", bufs=1, space="PSUM"))

        def body(_iv=None):
            # ---- warmup scratch + chain source ----
            wsrc = const.tile([P, 640], bf16, tag="wsrc", name="wsrc")
            nc.vector.memset(wsrc[:], 1.0)

            # ---- loads: few, ordered issues (each issue ~0.6us serial) ----
            xTr = const.tile([P, KS, T], bf16, tag="xT", name="xTr")
            Wr = {w: const.tile([P, KS, C], bf16, tag=f"W{w}", name=f"W{w}r")
                  for w in ("q", "k", "v", "p")}
            xT_r = xT_d.rearrange("(ks p) t -> p ks t", p=P)
            W_r = {w: W_d[w].rearrange("(ks p) c -> p ks c", p=P)
                   for w in ("q", "k", "v", "p")}
            nc.sync.dma_start(xTr[:, 0, :], xT_r[:, 0, :])
            nc.sync.dma_start(Wr["q"][:, 0, :], W_r["q"][:, 0, :])
            nc.sync.dma_start(xTr[:, 1:KS, :], xT_r[:, 1:KS, :])
            nc.sync.dma_start(Wr["q"][:, 1:KS, :], W_r["q"][:, 1:KS, :])
            nc.sync.dma_start(Wr["k"][:], W_r["k"][:])
            bqT = const.tile([P, KS], f32, tag="bqT", name="bqT")
            nc.sync.dma_start(bqT[:], bqT_d[:, :])
            bkT = const.tile([P, KS], f32, tag="bkT", name="bkT")
            nc.sync.dma_start(bkT[:], bkT_d[:, :])
            bvB = const.tile([P, C], f32, tag="bvB", name="bvB")
            nc.sync.dma_start(bvB[:], bvB_d[:, :])
            bpB = const.tile([P, C], f32, tag="bpB", name="bpB")
            nc.sync.dma_start(bpB[:], bpB_d[:, :])
            # Wv / Wp DMAs are gated: DMA transfers share bandwidth, so an
            # early Wv/Wp start would starve the critical-path Wk transfer.
            # A 2-element copy from QT/KT into the tile corner makes the DMA
            # wait (WAR) until the Q / K projections are done.

            # ---- PE warmup chain: keep HAM un-throttled during DMA wait.
            # Each link: 2 MMs then a DVE copy that the next link's rhs
            # region depends on, pacing ~1us/link without blocking DMAs.
            for wl in range(4):
                psw = psV.tile([P, 1024], f32, tag="psv", name="psw")
                nc.tensor.matmul(psw[:, 0:512], wsrc[:, 0:P],
                                 wsrc[:, P:P + 512], start=True, stop=True)
                nc.tensor.matmul(psw[:, 512:1024], wsrc[:, 0:P],
                                 wsrc[:, P:P + 512], start=True, stop=True)
                nc.vector.tensor_copy(wsrc[:, 128 + wl:129 + wl],
                                      psw[:, wl:wl + 1])

            ones1 = const.tile([P, 1], f32, tag="ones", name="ones1")
            nc.vector.memset(ones1[:], 1.0)
            V_aug = const.tile([P, TS, H, D + 1], bf16, tag="Vaug", name="Vaug")
            nc.vector.tensor_copy(V_aug[:, :, :, D:D + 1],
                                  ones1[:].to_broadcast([P, TS, H, 1]))

            YT = {k: const.tile([P, T], bf16, tag=f"YT{k}", name=f"YT{k}")
                  for k in range(KS)}
            YACC = const.tile([P, TS, C], f32, tag="YACC", name="YACC")

            # ---------------- emission helpers ----------------
            qk_state = {}

            def emit_qk_unit(p, w, ti, k):
                """One k-subtile of the QT/KT(p) ti-half projection. The
                [128,512] accumulator (psP1, single buffer) is allocated at
                k==0 and drained+biased at k==KS-1, so the four passes of a
                pair rotate through one bank without blocking out-chunks."""
                dst, bias = (QT, bqT) if w == "q" else (KT, bkT)
                if k == 0:
                    qk_state[(w, ti)] = psP1.tile([P, 512], f32, tag="psp1",
                                                  name="psp1")
                ps = qk_state[(w, ti)]
                nc.tensor.matmul(ps[:], Wr[w][:, k, p * P:(p + 1) * P],
                                 xTr[:, k, ti * 512:(ti + 1) * 512],
                                 start=(k == 0), stop=(k == KS - 1))
                if k == KS - 1:
                    nc.vector.tensor_tensor(
                        dst[p % 2][:, ti * 512:(ti + 1) * 512], ps[:],
                        bias[:, p:p + 1].to_broadcast([P, 512]), op=ALU.add)

            QK_UNITS = [(w, ti, k) for w, ti in
                        (("q", 0), ("q", 1), ("k", 0), ("k", 1))
                        for k in range(KS)]

            def emit_qk(p, w):
                for ti in range(NI):
                    for k in range(KS):
                        emit_qk_unit(p, w, ti, k)

            def emit_v(ts_):
                """V projection t-chunk into V_aug, one [128,1024] tile,
                one merged bias-add drain."""
                ps = psV.tile([P, 1024], f32, tag="psv", name="psva")
                for k in range(KS):
                    lhsT = xTr[:, k, ts_ * P:(ts_ + 1) * P]
                    nc.tensor.matmul(ps[:, 0:512], lhsT, Wr["v"][:, k, 0:512],
                                     start=(k == 0), stop=(k == KS - 1))
                    nc.tensor.matmul(ps[:, 512:768], lhsT, Wr["v"][:, k, 512:768],
                                     start=(k == 0), stop=(k == KS - 1))
                nc.vector.tensor_tensor(
                    V_aug[:, ts_, :, 0:D],
                    ps[:, 0:768].rearrange("p (h d) -> p h d", h=H),
                    bvB[:, 0:768].rearrange("p (h d) -> p h d", h=H),
                    op=ALU.add)

            def emit_s_exp(p, j, ptb):
                """S^T both heads for key-chunk j + one exp ACT.
                Per head: 1 LDW + 2 pipelined MMs; heads run concurrently
                on disjoint row groups (base partition 0 / 64)."""
                pst = {0: psS.tile([P, 1024], f32, tag="sA", name="sA"),
                       1: psS.tile([P, 1024], f32, tag="sB", name="sB")}
                for i in range(NI):
                    for hh in range(2):
                        b0 = 64 * hh
                        nc.tensor.matmul(
                            pst[i][:, hh * 512:(hh + 1) * 512],
                            KT[p % 2][b0:b0 + 64, j * P:(j + 1) * P],
                            QT[p % 2][b0:b0 + 64, i * 512:(i + 1) * 512],
                            start=True, stop=True)
                    # exp of query-half i for both heads; frees tile i for
                    # S(j+1) while the other half's ACT still runs.
                    nc.scalar.activation(ptb[:, j, i, :, :], pst[i][:], AF.Exp,
                                         scale=0.125)

            def emit_pv_j(p, hh, j, ptb, ps):
                """One key-chunk of the PV accumulation for head 2p+hh."""
                h = 2 * p + hh
                lhsT = V_aug[:, j, h, :]
                for i in range(NI):
                    nc.tensor.matmul(
                        ps[0:D + 1, i * 512:(i + 1) * 512], lhsT,
                        ptb[:, j, i, hh, :],
                        start=(j == 0), stop=(j == TS - 1))

            def emit_pv_norm(p, hh, ps, stage_scalar=False):
                """Normalize by the ones-row denominator into YT."""
                b0 = 64 * hh
                # stage the denominator row to SBUF first: the custom-DVE
                # reciprocal reads garbage from PSUM on real HW (sim is fine).
                # In the tail ScalarE is idle, so stage there instead of DVE.
                dd = npool.tile([1, 1024], f32, tag="dd", name="dd")
                if stage_scalar:
                    nc.scalar.copy(dd[0:1, :], ps[D:D + 1, :])
                else:
                    nc.vector.tensor_copy(dd[0:1, :], ps[D:D + 1, :])
                rr = npool.tile([1, 1024], f32, tag="rr", name="rr")
                nc.vector.reciprocal_approx_fast(rr[0:1, :], dd[0:1, :])
                rb = npool.tile([D, 1024], f32, tag="rb", name="rb")
                nc.gpsimd.partition_broadcast(rb[:], rr[0:1, :])
                nc.vector.tensor_tensor(YT[p][b0:b0 + 64, :], ps[0:D, :],
                                        rb[:], op=ALU.mult)

            def emit_pv_norm2(p, hh, psA, psB, stage_scalar=False):
                """Normalize a query-half-split PV accumulator pair."""
                b0 = 64 * hh
                dd = npool.tile([1, 1024], f32, tag="dd", name="dd")
                cp = nc.scalar.copy if stage_scalar else nc.vector.tensor_copy
                cp(dd[0:1, 0:512], psA[D:D + 1, :])
                cp(dd[0:1, 512:1024], psB[D:D + 1, :])
                rr = npool.tile([1, 1024], f32, tag="rr", name="rr")
                nc.vector.reciprocal_approx_fast(rr[0:1, :], dd[0:1, :])
                rb = npool.tile([D, 1024], f32, tag="rb", name="rb")
                nc.gpsimd.partition_broadcast(rb[:], rr[0:1, :])
                nc.vector.tensor_tensor(YT[p][b0:b0 + 64, 0:512],
                                        psA[0:D, :], rb[:, 0:512], op=ALU.mult)
                nc.vector.tensor_tensor(YT[p][b0:b0 + 64, 512:1024],
                                        psB[0:D, :], rb[:, 512:1024],
                                        op=ALU.mult)

            otile = {}

            def emit_out_half(ks_, ts_, half, pool=None, ps=None):
                """Half an output-projection chunk: one MM (cols 0-511 or
                512-767) over contraction subtiles ks_, one DVE add."""
                c0, c1 = (0, 512) if half == 0 else (512, 768)
                w_ = c1 - c0
                if ps is None:
                    pool = pool if pool is not None else psP2
                    tg = "psp1" if pool is psP1 else "pso"
                    ps = pool.tile([P, 512], f32, tag=tg, name="pso")
                for n, k in enumerate(ks_):
                    nc.tensor.matmul(ps[:, 0:w_],
                                     YT[k][:, ts_ * P:(ts_ + 1) * P],
                                     Wr["p"][:, k, c0:c1],
                                     start=(n == 0), stop=(n == len(ks_) - 1))
                first, last = (ks_[0] == 0), (ks_[-1] == KS - 1)
                if first:
                    nc.vector.tensor_tensor(YACC[:, ts_, c0:c1], ps[:, 0:w_],
                                            bpB[:, c0:c1], op=ALU.add)
                elif not last:
                    nc.vector.tensor_tensor(YACC[:, ts_, c0:c1],
                                            YACC[:, ts_, c0:c1], ps[:, 0:w_],
                                            op=ALU.add)
                else:
                    if half == 0:
                        otile[ts_] = opool.tile([P, C], bf16, tag="ot",
                                                name="ot")
                    ot = otile[ts_]
                    nc.vector.tensor_tensor(ot[:, c0:c1], YACC[:, ts_, c0:c1],
                                            ps[:, 0:w_], op=ALU.add)
                    if half == 1:
                        nc.sync.dma_start(y_d[ts_ * P:(ts_ + 1) * P, :], ot[:])

            # ---------------- prologue: QT/KT for pair 0 ----------------
            QT = {}
            KT = {}
            for pp in range(2):
                QT[pp] = const.tile([P, T], bf16, tag=f"QT{pp}", name="QT")
                KT[pp] = const.tile([P, T], bf16, tag=f"KT{pp}", name="KT")
            emit_qk(0, "q")
            emit_qk(0, "k")
            # Release the gated Wv / Wp loads now that Q/K are projected:
            # the corner copies below read QT/KT, so the DMAs (WAR on the
            # tiles) cannot start before the Q / K projections finish.
            nc.vector.tensor_copy(Wr["v"][0:1, 0, 0:2], QT[0][0:1, 0:2])
            nc.sync.dma_start(Wr["v"][:], W_r["v"][:])
            nc.vector.tensor_copy(Wr["p"][0:1, 0, 0:2], KT[0][0:1, 0:2])
            nc.sync.dma_start(Wr["p"][:], W_r["p"][:])

            # ---------------- pair loop ----------------
            ptbs = {pp: ptp.tile([P, TS, NI, 2, 512], bf16, tag=f"pt{pp}", name="pt")
                    for pp in range(2)}
            for p in range(NP):
                ptb = ptbs[p % 2]
                ptb_prev = ptbs[(p - 1) % 2]
                pv_tile = None
                for j in range(TS):
                    emit_s_exp(p, j, ptb)
                    if p == 0:
                        emit_v(j)
                        for u in range(3 * j, 3 * j + 3):
                            w, ti, k = QK_UNITS[u]
                            emit_qk_unit(1, w, ti, k)
                    else:
                        # PV(p-1): head0 at slots 1-3, head1 at 5-7 (3/3/2
                        # key-chunks). Slots 0 and 4 are handoff slack: the
                        # psV WAR on the previous head's normalize chain
                        # resolves before the next head's first piece, so
                        # stalled PV matmuls never head-of-line-block the S
                        # matmuls behind them in the TensorE FIFO.
                        PVJ = {1: (0, 1), 2: (2, 3, 4), 3: (5, 6, 7),
                               5: (0, 1), 6: (2, 3, 4), 7: (5, 6, 7)}
                        if j in PVJ:
                            hh = 0 if j < 4 else 1
                            if j in (1, 5):
                                pv_tile = psV.tile([P, 1024], f32, tag="psv",
                                                   name="psy")
                            for jj in PVJ[j]:
                                emit_pv_j(p - 1, hh, jj, ptb_prev, pv_tile)
                            if j in (3, 7):
                                emit_pv_norm(p - 1, hh, pv_tile)
                        if p < NP - 1:
                            for u in range(3 * j, 3 * j + 3):
                                w, ti, k = QK_UNITS[u]
                                emit_qk_unit(p + 1, w, ti, k)
                            if p >= 2:
                                emit_out_half((p - 2,), j, 0)
                                emit_out_half((p - 2,), j, 1)
                        else:
                            # Last pair: head0's PV runs j-SYNCED right
                            # behind its own exp (psP1/psP2 are free -- no
                            # QK(p+1) -- and the k3 output chunks moved to
                            # the tail), so the tail only carries head1.
                            if j == 0:
                                pv5 = {0: psP1.tile([P, 512], f32, tag="psp1",
                                                    name="pv5a"),
                                       1: psP2.tile([P, 512], f32, tag="pso",
                                                    name="pv5b")}
                            for i in range(NI):
                                nc.tensor.matmul(
                                    pv5[i][0:D + 1, :],
                                    V_aug[:, j, 2 * p, :], ptb[:, j, i, 0, :],
                                    start=(j == 0), stop=(j == TS - 1))

            # ---------------- tail ----------------
            # Head0 of the last pair already accumulated j-synced inside
            # pair 5 -- normalize it now; only head1's PV remains. Output
            # chunks are (k3,k4,k5) triples whose k3/k4 matmuls are ready
            # at (or just after) tail start and fill the PE while the
            # normalize chains run; eight half-buffers are borrowed from
            # the freed S / PV / projection banks.
            pl = ptbs[(NP - 1) % 2]
            emit_pv_norm2(NP - 1, 0, pv5[0], pv5[1], stage_scalar=True)
            t0 = psV.tile([P, 1024], f32, tag="psv", name="psy")
            for j in range(TS):
                emit_pv_j(NP - 1, 1, j, pl, t0)
            emit_pv_norm(NP - 1, 1, t0, stage_scalar=True)

            sA = psS.tile([P, 1024], f32, tag="sA", name="psoA")
            sB = psS.tile([P, 1024], f32, tag="sB", name="psoB")
            tail_ps = {}
            KTRI = (KS - 3, KS - 2, KS - 1)
            for wave in range(2):
                tss = range(wave * 4, wave * 4 + 4)
                for ts_ in tss:
                    for half in range(2):
                        idx = (ts_ % 4) * 2 + half
                        if idx < 4:
                            hb = sA if idx < 2 else sB
                            ps = hb[:, (idx % 2) * 512:(idx % 2) * 512 + 512]
                        elif idx < 6:
                            pool = psP1 if idx == 4 else psP2
                            ps = pool.tile([P, 512], f32,
                                           tag="psp1" if idx == 4 else "pso",
                                           name="pso")
                        else:
                            ps = t0[:, (idx % 2) * 512:(idx % 2) * 512 + 512]
                        tail_ps[(ts_, half)] = ps
                        c0, c1 = (0, 512) if half == 0 else (512, 768)
                        for n, k in enumerate(KTRI[:2]):
                            nc.tensor.matmul(
                                ps[:, 0:c1 - c0],
                                YT[k][:, ts_ * P:(ts_ + 1) * P],
                                Wr["p"][:, k, c0:c1],
                                start=(n == 0), stop=False)
                for ts_ in tss:
                    for half in range(2):
                        ps = tail_ps[(ts_, half)]
                        c0, c1 = (0, 512) if half == 0 else (512, 768)
                        nc.tensor.matmul(ps[:, 0:c1 - c0],
                                         YT[KS - 1][:, ts_ * P:(ts_ + 1) * P],
                                         Wr["p"][:, KS - 1, c0:c1],
                                         start=False, stop=True)
                        if half == 0:
                            otile[ts_] = opool.tile([P, C], bf16, tag="ot",
                                                    name="ot")
                        ot = otile[ts_]
                        nc.vector.tensor_tensor(ot[:, c0:c1],
                                                YACC[:, ts_, c0:c1],
                                                ps[:, 0:c1 - c0], op=ALU.add)
                        if half == 1:
                            nc.sync.dma_start(y_d[ts_ * P:(ts_ + 1) * P, :],
                                              ot[:])

        if reps == 1:
            body()
        else:
            import concourse.mybir as _mb
            with tc.For_i(0, reps, 1, hint_engines=tuple(_mb.ALL_ENGINES)):
                body()

    nc.compile()
    return nc


class _Runner:
    """Compile once, run many times on the 8 axon-tunneled cores via PJRT."""

    def __init__(self, nc, n_cores):
        import jax
        import concourse.mybir as mybir
        from jax.sharding import Mesh, PartitionSpec
        from jax.experimental.shard_map import shard_map
        from concourse.bass2jax import (
            _bass_exec_p, install_neuronx_cc_hook, partition_id_tensor)

        install_neuronx_cc_hook()
        self.jax = jax
        self.n_cores = n_cores
        partition_name = nc.partition_id_tensor.name if nc.partition_id_tensor else None
        in_names, out_names, out_avals, zero_outs = [], [], [], []
        for alloc in nc.m.functions[0].allocations:
            if not isinstance(alloc, mybir.MemoryLocationSet):
                continue
            name = alloc.memorylocations[0].name
            if alloc.kind == "ExternalInput":
                if name != partition_name:
                    in_names.append(name)
            elif alloc.kind == "ExternalOutput":
                shape = tuple(alloc.tensor_shape)
                dtype = mybir.dt.np(alloc.dtype)
                out_names.append(name)
                out_avals.append(jax.core.ShapedArray(shape, dtype))
                zero_outs.append(np.zeros(shape, dtype))
        self.in_names, self.out_names = in_names, out_names
        self.zero_outs = zero_outs
        all_in = list(in_names) + list(out_names)
        if partition_name is not None:
            all_in.append(partition_name)

        def _body(*args):
            operands = list(args)
            if partition_name is not None:
                operands.append(partition_id_tensor())
            return tuple(_bass_exec_p.bind(
                *operands, out_avals=tuple(out_avals), in_names=tuple(all_in),
                out_names=tuple(out_names), lowering_input_output_aliases=(),
                sim_require_finite=True, sim_require_nnan=True, nc=nc))

        devices = jax.devices()[:n_cores]
        self.mesh = Mesh(np.asarray(devices), ("core",))
        spec = PartitionSpec("core")
        self.fn = jax.jit(
            shard_map(_body, mesh=self.mesh,
                      in_specs=(spec,) * (len(in_names) + len(out_names)),
                      out_specs=(spec,) * len(out_names), check_rep=False),
            keep_unused=True)

    def stage(self, in_maps):
        import jax
        from jax.sharding import PartitionSpec
        concat = [
            np.concatenate([np.asarray(in_maps[c][n]) for c in range(self.n_cores)], axis=0)
            for n in self.in_names
        ] + [np.concatenate([z] * self.n_cores, axis=0) for z in self.zero_outs]
        sharding = jax.sharding.NamedSharding(self.mesh, PartitionSpec("core"))
        return [jax.device_put(a, sharding) for a in concat]

    def run(self, staged):
        outs = self.fn(*staged)
        self.jax.block_until_ready(outs)
        return outs

    def run_to_maps(self, staged):
        outs = self.run(staged)
        res = []
        for c in range(self.n_cores):
            m = {}
            for i, n in enumerate(self.out_names):
                g = np.asarray(outs[i])
                per = g.shape[0] // self.n_cores
                m[n] = g[c * per:(c + 1) * per]
            res.append(m)
        return res


def get_runner(reps: int = 1, phases: int = 4, variant: str = "full"):
    key = (reps, phases, variant)
    if key not in _RUNNER_CACHE:
        nc = build_nc(reps, phases, variant)
        _RUNNER_CACHE[key] = _Runner(nc, N_CORES)
    return _RUNNER_CACHE[key]


def make_in_maps(x, Wq, bq, Wk, bk, Wv, bv, Wp, bp):
    import ml_dtypes
    bf = ml_dtypes.bfloat16
    x = np.asarray(x, dtype=np.float32)
    weights = {
        "Wq": np.asarray(Wq, bf), "Wk": np.asarray(Wk, bf),
        "Wv": np.asarray(Wv, bf), "Wp": np.asarray(Wp, bf),
    }
    bqT = np.ascontiguousarray(np.asarray(bq, np.float32).reshape(KS, P).T)
    bkT = np.ascontiguousarray(np.asarray(bk, np.float32).reshape(KS, P).T)
    bvB = np.ascontiguousarray(np.broadcast_to(np.asarray(bv, np.float32), (P, C)))
    bpB = np.ascontiguousarray(np.broadcast_to(np.asarray(bp, np.float32), (P, C)))
    in_maps = []
    for b in range(B):
        in_maps.append({
            "xT": np.ascontiguousarray(x[b].T.astype(bf)),
            "Wq": weights["Wq"], "Wk": weights["Wk"],
            "Wv": weights["Wv"], "Wp": weights["Wp"],
            "bqT": bqT, "bkT": bkT, "bvB": bvB, "bpB": bpB,
        })
    return in_maps


def kernel(x, Wq, bq, Wk, bk, Wv, bv, Wp, bp):
    runner = get_runner(reps=1)
    in_maps = make_in_maps(x, Wq, bq, Wk, bk, Wv, bv, Wp, bp)
    staged = runner.stage(in_maps)
    res = runner.run_to_maps(staged)
    return np.stack([np.asarray(res[b]["y"], np.float32) for b in range(B)],
                    axis=0)


# revision 31
# speedup vs baseline: 1.4843x; 1.2020x over previous
"""Multi-head attention forward on 8 Trainium2 NeuronCores (Bass/Tile).

Problem: nn_MultiHeadAttention — B=8, T=1024, C=768, H=12, D=64, fp32 in/out.

Sharding: data-parallel over batch — one batch element per core; weights
broadcast. No collectives. Host pre-transposes x[b] to x^T [C, T], converts
to bf16, and upcasts the bf16 output back to fp32.

Design (vs the f32r baseline: single-shot ~223us -> ~220us, steady-state
slope 166us -> ~135us per iteration):
  - bf16 data path (PSUM fp32): halves HBM bytes and DVE element time and
    enables fast weight load; rel err ~6.5e-3 (tolerance 2e-2).
  - S^T per head is a K=64 contraction: the two heads of a pair run as
    row-tiled CONCURRENT matmuls (head0 at partitions 0-63 -> row groups
    0-1, head1 at 64-127; tile_position auto-derives from base_partition),
    so no zero-padded K=128 tiles and no 2x MAC waste.
  - Exp runs as TWO [128,1024] ACTs per (pair, key-chunk j), one per query
    half on its own 2-bank PSUM tile (tags sA/sB): S(j+1)'s first matmuls
    only wait the first half's ACT, which shortens the serial S->exp chain
    while ScalarE stays ~saturated (96 ACTs x 1.15us = 110us).
  - PV is deferred one pair (pt kept in SBUF [P,TS,2,2,512]) and consumed
    during the next pair's S/exp phase: [65,1024] accumulators with the
    ones-column denominator trick; one normalize chain per (pair, head).
    PV pieces sit at slots 1-3/5-7 (2/3/3 chunks) so the psV
    write-after-read handoff on the previous head's normalize never
    head-of-line-blocks the S matmuls behind them in the TensorE FIFO.
    The LAST pair's head0 PV runs j-synced inside its own pair on the
    freed projection banks, so the tail only carries head1 + the k3-k5
    output chunks (k3/k4 matmuls are ready at tail start).
  - YT is six per-k tiles, not one [P,KS,T] tile: the Tile region tracker
    coarsens 3D-strided slices to whole-tile, which made every output-
    projection matmul falsely depend on every PV normalize.
  - V projection runs inside pair0's slots (per-chunk dependency); QK
    projections are emitted as per-k units, 3 per slot; the output
    projection is spread across pairs as per-k half-chunks accumulated in
    SBUF fp32, with k=4,5 in a two-wave tail on 8 borrowed PSUM buffers.
  - DMA: few big issues ordered xT/Wq/Wk first; Wv and Wp transfers are
    GATED behind Q/K projection completion via a tile-corner copy (DMA
    transfers share bandwidth, so early Wv/Wp starts would starve the
    critical-path Wk transfer).
  - A 4-link PE warmup chain runs during the DMA prologue so the HAM clock
    gate is released (2.4 GHz) before real matmuls start.
  - PSUM: S 2+2 banks + V/PV 2 + projection 1 + output 1 = 8.
  - Known HW quirk: custom-DVE reciprocal_approx_fast reads garbage from
    PSUM on hardware (sim is fine) -> denominator row is staged to SBUF.
"""
import numpy as np

B, T, C = 8, 1024, 768
H, D = 12, 64
P = 128
KS = C // P          # 6 contraction subtiles
TS = 8               # 8 t subtiles
NP = H // 2          # 6 head pairs
NI = 2
N_CORES = 8

_RUNNER_CACHE = {}


def build_nc(reps: int = 1, phases: int = 4, variant: str = "full"):
    import concourse.bacc as bacc
    import concourse.mybir as mybir
    import concourse.tile as tile
    from contextlib import ExitStack

    f32 = mybir.dt.float32
    bf16 = mybir.dt.bfloat16
    AF = mybir.ActivationFunctionType
    ALU = mybir.AluOpType

    nc = bacc.Bacc(num_devices=N_CORES)

    xT_d = nc.dram_tensor("xT", [C, T], bf16, kind="ExternalInput")
    W_d = {w: nc.dram_tensor(f"W{w}", [C, C], bf16, kind="ExternalInput")
           for w in ("q", "k", "v", "p")}
    bqT_d = nc.dram_tensor("bqT", [P, KS], f32, kind="ExternalInput")
    bkT_d = nc.dram_tensor("bkT", [P, KS], f32, kind="ExternalInput")
    bvB_d = nc.dram_tensor("bvB", [P, C], f32, kind="ExternalInput")
    bpB_d = nc.dram_tensor("bpB", [P, C], f32, kind="ExternalInput")
    y_d = nc.dram_tensor("y", [T, C], bf16, kind="ExternalOutput")

    with tile.TileContext(nc) as tc, ExitStack() as ctx:
        const = ctx.enter_context(tc.tile_pool(name="const", bufs=1))
        ptp = ctx.enter_context(tc.tile_pool(name="ptp", bufs=1))
        npool = ctx.enter_context(tc.tile_pool(name="norm", bufs=2))
        opool = ctx.enter_context(tc.tile_pool(name="out", bufs=2))
        psS = ctx.enter_context(tc.tile_pool(name="psS", bufs=1, space="PSUM"))
        psV = ctx.enter_context(tc.tile_pool(name="psV", bufs=1, space="PSUM"))
        psP1 = ctx.enter_context(tc.tile_pool(name="psP1", bufs=1, space="PSUM"))
        psP2 = ctx.enter_context(tc.tile_pool(name="psP2", bufs=1, space="PSUM"))

        def body(_iv=None):
            # ---- warmup scratch + chain source ----
            wsrc = const.tile([P, 640], bf16, tag="wsrc", name="wsrc")
            nc.vector.memset(wsrc[:], 1.0)

            # ---- loads: few, ordered issues (each issue ~0.6us serial) ----
            xTr = const.tile([P, KS, T], bf16, tag="xT", name="xTr")
            Wr = {w: const.tile([P, KS, C], bf16, tag=f"W{w}", name=f"W{w}r")
                  for w in ("q", "k", "v", "p")}
            xT_r = xT_d.rearrange("(ks p) t -> p ks t", p=P)
            W_r = {w: W_d[w].rearrange("(ks p) c -> p ks c", p=P)
                   for w in ("q", "k", "v", "p")}
            nc.sync.dma_start(xTr[:, 0, :], xT_r[:, 0, :])
            nc.sync.dma_start(Wr["q"][:, 0, :], W_r["q"][:, 0, :])
            nc.sync.dma_start(xTr[:, 1:KS, :], xT_r[:, 1:KS, :])
            nc.sync.dma_start(Wr["q"][:, 1:KS, :], W_r["q"][:, 1:KS, :])
            nc.sync.dma_start(Wr["k"][:], W_r["k"][:])
            bqT = const.tile([P, KS], f32, tag="bqT", name="bqT")
            nc.sync.dma_start(bqT[:], bqT_d[:, :])
            bkT = const.tile([P, KS], f32, tag="bkT", name="bkT")
            nc.sync.dma_start(bkT[:], bkT_d[:, :])
            bvB = const.tile([P, C], f32, tag="bvB", name="bvB")
            nc.sync.dma_start(bvB[:], bvB_d[:, :])
            bpB = const.tile([P, C], f32, tag="bpB", name="bpB")
            nc.sync.dma_start(bpB[:], bpB_d[:, :])
            # Wv / Wp DMAs are gated: DMA transfers share bandwidth, so an
            # early Wv/Wp start would starve the critical-path Wk transfer.
            # A 2-element copy from QT/KT into the tile corner makes the DMA
            # wait (WAR) until the Q / K projections are done.

            # ---- PE warmup chain: keep HAM un-throttled during DMA wait.
            # Each link: 2 MMs then a DVE copy that the next link's rhs
            # region depends on, pacing ~1us/link without blocking DMAs.
            for wl in range(3):
                psw = psV.tile([P, 1024], f32, tag="psv", name="psw")
                nc.tensor.matmul(psw[:, 0:512], wsrc[:, 0:P],
                                 wsrc[:, P:P + 512], start=True, stop=True)
                nc.tensor.matmul(psw[:, 512:1024], wsrc[:, 0:P],
                                 wsrc[:, P:P + 512], start=True, stop=True)
                nc.vector.tensor_copy(wsrc[:, 128 + wl:129 + wl],
                                      psw[:, wl:wl + 1])

            ones1 = const.tile([P, 1], f32, tag="ones", name="ones1")
            nc.vector.memset(ones1[:], 1.0)
            V_aug = const.tile([P, TS, H, D + 1], bf16, tag="Vaug", name="Vaug")
            nc.vector.tensor_copy(V_aug[:, :, :, D:D + 1],
                                  ones1[:].to_broadcast([P, TS, H, 1]))

            YT = {k: const.tile([P, T], bf16, tag=f"YT{k}", name=f"YT{k}")
                  for k in range(KS)}
            YACC = const.tile([P, TS, C], f32, tag="YACC", name="YACC")

            # ---------------- emission helpers ----------------
            qk_state = {}

            def emit_qk_unit(p, w, ti, k):
                """One k-subtile of the QT/KT(p) ti-half projection. The
                [128,512] accumulator (psP1, single buffer) is allocated at
                k==0 and drained+biased at k==KS-1, so the four passes of a
                pair rotate through one bank without blocking out-chunks."""
                dst, bias = (QT, bqT) if w == "q" else (KT, bkT)
                if k == 0:
                    qk_state[(w, ti)] = psP1.tile([P, 512], f32, tag="psp1",
                                                  name="psp1")
                ps = qk_state[(w, ti)]
                nc.tensor.matmul(ps[:], Wr[w][:, k, p * P:(p + 1) * P],
                                 xTr[:, k, ti * 512:(ti + 1) * 512],
                                 start=(k == 0), stop=(k == KS - 1))
                if k == KS - 1:
                    nc.vector.tensor_tensor(
                        dst[p % 2][:, ti * 512:(ti + 1) * 512], ps[:],
                        bias[:, p:p + 1].to_broadcast([P, 512]), op=ALU.add)

            QK_UNITS = [(w, ti, k) for w, ti in
                        (("q", 0), ("q", 1), ("k", 0), ("k", 1))
                        for k in range(KS)]

            qkp_state = {}

            def emit_qk_pair_k(p, w, k):
                """One k-subtile of the dual-bank QK pass (both ti halves,
                shared LDWEIGHTS), spreadable across slots."""
                dst, bias = (QT, bqT) if w == "q" else (KT, bkT)
                if k == 0:
                    qkp_state[w] = (
                        psP1.tile([P, 512], f32, tag="psp1", name="pspA"),
                        psP2.tile([P, 512], f32, tag="pso", name="pspB"))
                psA, psB = qkp_state[w]
                lhsT = Wr[w][:, k, p * P:(p + 1) * P]
                nc.tensor.matmul(psA[:], lhsT, xTr[:, k, 0:512],
                                 start=(k == 0), stop=(k == KS - 1))
                nc.tensor.matmul(psB[:], lhsT, xTr[:, k, 512:1024],
                                 start=(k == 0), stop=(k == KS - 1))
                if k == KS - 1:
                    nc.vector.tensor_tensor(
                        dst[p % 2][:, 0:512], psA[:],
                        bias[:, p:p + 1].to_broadcast([P, 512]), op=ALU.add)
                    nc.vector.tensor_tensor(
                        dst[p % 2][:, 512:1024], psB[:],
                        bias[:, p:p + 1].to_broadcast([P, 512]), op=ALU.add)

            def emit_qk_pair(p, w):
                """Both ti-half accumulators at once on psP1+psP2 (psP2 is
                idle before pair 2): one LDW + two pipelined MMs per k —
                halves the projection critical path in the prologue/pair0."""
                dst, bias = (QT, bqT) if w == "q" else (KT, bkT)
                psA = psP1.tile([P, 512], f32, tag="psp1", name="pspA")
                psB = psP2.tile([P, 512], f32, tag="pso", name="pspB")
                for k in range(KS):
                    lhsT = Wr[w][:, k, p * P:(p + 1) * P]
                    nc.tensor.matmul(psA[:], lhsT, xTr[:, k, 0:512],
                                     start=(k == 0), stop=(k == KS - 1))
                    nc.tensor.matmul(psB[:], lhsT, xTr[:, k, 512:1024],
                                     start=(k == 0), stop=(k == KS - 1))
                nc.vector.tensor_tensor(
                    dst[p % 2][:, 0:512], psA[:],
                    bias[:, p:p + 1].to_broadcast([P, 512]), op=ALU.add)
                nc.vector.tensor_tensor(
                    dst[p % 2][:, 512:1024], psB[:],
                    bias[:, p:p + 1].to_broadcast([P, 512]), op=ALU.add)

            def emit_qk(p, w):
                for ti in range(NI):
                    for k in range(KS):
                        emit_qk_unit(p, w, ti, k)

            def emit_v(ts_):
                """V projection t-chunk into V_aug, one [128,1024] tile,
                one merged bias-add drain."""
                ps = psV.tile([P, 1024], f32, tag="psv", name="psva")
                for k in range(KS):
                    lhsT = xTr[:, k, ts_ * P:(ts_ + 1) * P]
                    nc.tensor.matmul(ps[:, 0:512], lhsT, Wr["v"][:, k, 0:512],
                                     start=(k == 0), stop=(k == KS - 1))
                    nc.tensor.matmul(ps[:, 512:768], lhsT, Wr["v"][:, k, 512:768],
                                     start=(k == 0), stop=(k == KS - 1))
                nc.vector.tensor_tensor(
                    V_aug[:, ts_, :, 0:D],
                    ps[:, 0:768].rearrange("p (h d) -> p h d", h=H),
                    bvB[:, 0:768].rearrange("p (h d) -> p h d", h=H),
                    op=ALU.add)

            def emit_s_exp(p, j, ptb):
                """S^T both heads for key-chunk j + one exp ACT.
                Per head: 1 LDW + 2 pipelined MMs; heads run concurrently
                on disjoint row groups (base partition 0 / 64)."""
                pst = {0: psS.tile([P, 1024], f32, tag="sA", name="sA"),
                       1: psS.tile([P, 1024], f32, tag="sB", name="sB")}
                for i in range(NI):
                    for hh in range(2):
                        b0 = 64 * hh
                        nc.tensor.matmul(
                            pst[i][:, hh * 512:(hh + 1) * 512],
                            KT[p % 2][b0:b0 + 64, j * P:(j + 1) * P],
                            QT[p % 2][b0:b0 + 64, i * 512:(i + 1) * 512],
                            start=True, stop=True)
                    # exp of query-half i for both heads; frees tile i for
                    # S(j+1) while the other half's ACT still runs.
                    nc.scalar.activation(ptb[:, j, i, :, :], pst[i][:], AF.Exp,
                                         scale=0.125)

            def emit_pv_j(p, hh, j, ptb, ps):
                """One key-chunk of the PV accumulation for head 2p+hh."""
                h = 2 * p + hh
                lhsT = V_aug[:, j, h, :]
                for i in range(NI):
                    nc.tensor.matmul(
                        ps[0:D + 1, i * 512:(i + 1) * 512], lhsT,
                        ptb[:, j, i, hh, :],
                        start=(j == 0), stop=(j == TS - 1))

            def emit_pv_norm(p, hh, ps, stage_scalar=False):
                """Normalize by the ones-row denominator into YT.

                The PSUM accumulator is released after ~2us (denominator row
                + numerator copied out first); the reciprocal/broadcast/
                multiply then run SBUF-side off the critical path, so the
                next head's PV allocation never stalls on this chain."""
                b0 = 64 * hh
                # stage to SBUF first: the custom-DVE reciprocal reads
                # garbage from PSUM on real HW (sim is fine). In the tail
                # ScalarE is idle, so stage the denominator there.
                dd = npool.tile([1, 1024], f32, tag="dd", name="dd")
                if stage_scalar:
                    nc.scalar.copy(dd[0:1, :], ps[D:D + 1, :])
                else:
                    nc.vector.tensor_copy(dd[0:1, :], ps[D:D + 1, :])
                yn = npool.tile([D, 1024], bf16, tag="yn", name="yn")
                nc.vector.tensor_copy(yn[:], ps[0:D, :])
                rr = npool.tile([1, 1024], f32, tag="rr", name="rr")
                nc.vector.reciprocal_approx_fast(rr[0:1, :], dd[0:1, :])
                rb = npool.tile([D, 1024], f32, tag="rb", name="rb")
                nc.gpsimd.partition_broadcast(rb[:], rr[0:1, :])
                nc.vector.tensor_tensor(YT[p][b0:b0 + 64, :], yn[:],
                                        rb[:], op=ALU.mult)

            def emit_pv_norm2(p, hh, psA, psB, stage_scalar=False):
                """Normalize a query-half-split PV accumulator pair."""
                b0 = 64 * hh
                dd = npool.tile([1, 1024], f32, tag="dd", name="dd")
                cp = nc.scalar.copy if stage_scalar else nc.vector.tensor_copy
                cp(dd[0:1, 0:512], psA[D:D + 1, :])
                cp(dd[0:1, 512:1024], psB[D:D + 1, :])
                rr = npool.tile([1, 1024], f32, tag="rr", name="rr")
                nc.vector.reciprocal_approx_fast(rr[0:1, :], dd[0:1, :])
                rb = npool.tile([D, 1024], f32, tag="rb", name="rb")
                nc.gpsimd.partition_broadcast(rb[:], rr[0:1, :])
                nc.vector.tensor_tensor(YT[p][b0:b0 + 64, 0:512],
                                        psA[0:D, :], rb[:, 0:512], op=ALU.mult)
                nc.vector.tensor_tensor(YT[p][b0:b0 + 64, 512:1024],
                                        psB[0:D, :], rb[:, 512:1024],
                                        op=ALU.mult)

            otile = {}

            def emit_out_half(ks_, ts_, half, pool=None, ps=None):
                """Half an output-projection chunk: one MM (cols 0-511 or
                512-767) over contraction subtiles ks_, one DVE add."""
                c0, c1 = (0, 512) if half == 0 else (512, 768)
                w_ = c1 - c0
                if ps is None:
                    pool = pool if pool is not None else psP2
                    tg = "psp1" if pool is psP1 else "pso"
                    ps = pool.tile([P, 512], f32, tag=tg, name="pso")
                for n, k in enumerate(ks_):
                    nc.tensor.matmul(ps[:, 0:w_],
                                     YT[k][:, ts_ * P:(ts_ + 1) * P],
                                     Wr["p"][:, k, c0:c1],
                                     start=(n == 0), stop=(n == len(ks_) - 1))
                first, last = (ks_[0] == 0), (ks_[-1] == KS - 1)
                if first:
                    nc.vector.tensor_tensor(YACC[:, ts_, c0:c1], ps[:, 0:w_],
                                            bpB[:, c0:c1], op=ALU.add)
                elif not last:
                    nc.vector.tensor_tensor(YACC[:, ts_, c0:c1],
                                            YACC[:, ts_, c0:c1], ps[:, 0:w_],
                                            op=ALU.add)
                else:
                    if half == 0:
                        otile[ts_] = opool.tile([P, C], bf16, tag="ot",
                                                name="ot")
                    ot = otile[ts_]
                    nc.vector.tensor_tensor(ot[:, c0:c1], YACC[:, ts_, c0:c1],
                                            ps[:, 0:w_], op=ALU.add)
                    if half == 1:
                        nc.sync.dma_start(y_d[ts_ * P:(ts_ + 1) * P, :], ot[:])

            # ---------------- prologue: QT/KT for pair 0 ----------------
            QT = {}
            KT = {}
            for pp in range(2):
                QT[pp] = const.tile([P, T], bf16, tag=f"QT{pp}", name="QT")
                KT[pp] = const.tile([P, T], bf16, tag=f"KT{pp}", name="KT")
            emit_qk_pair(0, "q")
            emit_qk_pair(0, "k")
            # Release the gated Wv / Wp loads now that Q/K are projected:
            # the corner copies below read QT/KT, so the DMAs (WAR on the
            # tiles) cannot start before the Q / K projections finish.
            nc.vector.tensor_copy(Wr["v"][0:1, 0, 0:2], QT[0][0:1, 0:2])
            nc.sync.dma_start(Wr["v"][:], W_r["v"][:])
            nc.vector.tensor_copy(Wr["p"][0:1, 0, 0:2], KT[0][0:1, 0:2])
            nc.sync.dma_start(Wr["p"][:], W_r["p"][:])

            # ---------------- pair loop ----------------
            ptbs = {pp: ptp.tile([P, TS, NI, 2, 512], bf16, tag=f"pt{pp}", name="pt")
                    for pp in range(2)}
            for p in range(NP):
                ptb = ptbs[p % 2]
                ptb_prev = ptbs[(p - 1) % 2]
                pv_tile = None
                for j in range(TS):
                    emit_s_exp(p, j, ptb)
                    if p == 0:
                        emit_v(j)
                        if 1 <= j <= 3:
                            for kk in (2 * (j - 1), 2 * j - 1):
                                emit_qk_pair_k(1, "q", kk)
                        elif 4 <= j <= 6:
                            for kk in (2 * (j - 4), 2 * (j - 4) + 1):
                                emit_qk_pair_k(1, "k", kk)
                    else:
                        # PV(p-1): head0 at slots 1-3, head1 at 5-7 (3/3/2
                        # key-chunks). Slots 0 and 4 are handoff slack: the
                        # psV WAR on the previous head's normalize chain
                        # resolves before the next head's first piece, so
                        # stalled PV matmuls never head-of-line-block the S
                        # matmuls behind them in the TensorE FIFO.
                        PVJ = {1: (0, 1), 2: (2, 3, 4), 3: (5, 6, 7),
                               4: (0, 1), 5: (2, 3, 4), 6: (5, 6, 7)}
                        if j in PVJ:
                            hh = 0 if j < 4 else 1
                            if j in (1, 4):
                                pv_tile = psV.tile([P, 1024], f32, tag="psv",
                                                   name="psy")
                            for jj in PVJ[j]:
                                emit_pv_j(p - 1, hh, jj, ptb_prev, pv_tile)
                            if j in (3, 6):
                                emit_pv_norm(p - 1, hh, pv_tile)
                        if p < NP - 1:
                            # 4 units/slot over slots 0-5: the KT ti1 drain
                            # lands ~2 slots before the pair boundary, so
                            # S(p+1, j=0) never waits the projection.
                            if j < 6:
                                for u in range(4 * j, 4 * j + 4):
                                    w, ti, k = QK_UNITS[u]
                                    emit_qk_unit(p + 1, w, ti, k)
                            if p >= 2:
                                emit_out_half((p - 2,), j, 0)
                                emit_out_half((p - 2,), j, 1)
                        else:
                            # Last pair: head0's PV runs j-SYNCED right
                            # behind its own exp (psP1/psP2 are free -- no
                            # QK(p+1) -- and the k3 output chunks moved to
                            # the tail), so the tail only carries head1.
                            if j == 0:
                                pv5 = {0: psP1.tile([P, 512], f32, tag="psp1",
                                                    name="pv5a"),
                                       1: psP2.tile([P, 512], f32, tag="pso",
                                                    name="pv5b")}
                            for i in range(NI):
                                nc.tensor.matmul(
                                    pv5[i][0:D + 1, :],
                                    V_aug[:, j, 2 * p, :], ptb[:, j, i, 0, :],
                                    start=(j == 0), stop=(j == TS - 1))

            # ---------------- tail ----------------
            # Head0 of the last pair already accumulated j-synced inside
            # pair 5 -- normalize it now; only head1's PV remains. Output
            # chunks are (k3,k4,k5) triples whose k3/k4 matmuls are ready
            # at (or just after) tail start and fill the PE while the
            # normalize chains run; eight half-buffers are borrowed from
            # the freed S / PV / projection banks.
            pl = ptbs[(NP - 1) % 2]
            emit_pv_norm2(NP - 1, 0, pv5[0], pv5[1], stage_scalar=True)
            t0 = psV.tile([P, 1024], f32, tag="psv", name="psy")
            for j in range(TS):
                emit_pv_j(NP - 1, 1, j, pl, t0)
            emit_pv_norm(NP - 1, 1, t0, stage_scalar=True)

            sA = psS.tile([P, 1024], f32, tag="sA", name="psoA")
            sB = psS.tile([P, 1024], f32, tag="sB", name="psoB")
            tail_ps = {}
            KTRI = (KS - 3, KS - 2, KS - 1)
            for wave in range(2):
                tss = range(wave * 4, wave * 4 + 4)
                for ts_ in tss:
                    for half in range(2):
                        idx = (ts_ % 4) * 2 + half
                        if idx < 4:
                            hb = sA if idx < 2 else sB
                            ps = hb[:, (idx % 2) * 512:(idx % 2) * 512 + 512]
                        elif idx < 6:
                            pool = psP1 if idx == 4 else psP2
                            ps = pool.tile([P, 512], f32,
                                           tag="psp1" if idx == 4 else "pso",
                                           name="pso")
                        else:
                            ps = t0[:, (idx % 2) * 512:(idx % 2) * 512 + 512]
                        tail_ps[(ts_, half)] = ps
                        c0, c1 = (0, 512) if half == 0 else (512, 768)
                        for n, k in enumerate(KTRI[:2]):
                            nc.tensor.matmul(
                                ps[:, 0:c1 - c0],
                                YT[k][:, ts_ * P:(ts_ + 1) * P],
                                Wr["p"][:, k, c0:c1],
                                start=(n == 0), stop=False)
                for ts_ in tss:
                    for half in range(2):
                        ps = tail_ps[(ts_, half)]
                        c0, c1 = (0, 512) if half == 0 else (512, 768)
                        nc.tensor.matmul(ps[:, 0:c1 - c0],
                                         YT[KS - 1][:, ts_ * P:(ts_ + 1) * P],
                                         Wr["p"][:, KS - 1, c0:c1],
                                         start=False, stop=True)
                        if half == 0:
                            otile[ts_] = opool.tile([P, C], bf16, tag="ot",
                                                    name="ot")
                        ot = otile[ts_]
                        nc.vector.tensor_tensor(ot[:, c0:c1],
                                                YACC[:, ts_, c0:c1],
                                                ps[:, 0:c1 - c0], op=ALU.add)
                        if half == 1:
                            nc.sync.dma_start(y_d[ts_ * P:(ts_ + 1) * P, :],
                                              ot[:])

        if reps == 1:
            body()
        else:
            import concourse.mybir as _mb
            with tc.For_i(0, reps, 1, hint_engines=tuple(_mb.ALL_ENGINES)):
                body()

    nc.compile()
    return nc


class _Runner:
    """Compile once, run many times on the 8 axon-tunneled cores via PJRT."""

    def __init__(self, nc, n_cores):
        import jax
        import concourse.mybir as mybir
        from jax.sharding import Mesh, PartitionSpec
        from jax.experimental.shard_map import shard_map
        from concourse.bass2jax import (
            _bass_exec_p, install_neuronx_cc_hook, partition_id_tensor)

        install_neuronx_cc_hook()
        self.jax = jax
        self.n_cores = n_cores
        partition_name = nc.partition_id_tensor.name if nc.partition_id_tensor else None
        in_names, out_names, out_avals, zero_outs = [], [], [], []
        for alloc in nc.m.functions[0].allocations:
            if not isinstance(alloc, mybir.MemoryLocationSet):
                continue
            name = alloc.memorylocations[0].name
            if alloc.kind == "ExternalInput":
                if name != partition_name:
                    in_names.append(name)
            elif alloc.kind == "ExternalOutput":
                shape = tuple(alloc.tensor_shape)
                dtype = mybir.dt.np(alloc.dtype)
                out_names.append(name)
                out_avals.append(jax.core.ShapedArray(shape, dtype))
                zero_outs.append(np.zeros(shape, dtype))
        self.in_names, self.out_names = in_names, out_names
        self.zero_outs = zero_outs
        all_in = list(in_names) + list(out_names)
        if partition_name is not None:
            all_in.append(partition_name)

        def _body(*args):
            operands = list(args)
            if partition_name is not None:
                operands.append(partition_id_tensor())
            return tuple(_bass_exec_p.bind(
                *operands, out_avals=tuple(out_avals), in_names=tuple(all_in),
                out_names=tuple(out_names), lowering_input_output_aliases=(),
                sim_require_finite=True, sim_require_nnan=True, nc=nc))

        devices = jax.devices()[:n_cores]
        self.mesh = Mesh(np.asarray(devices), ("core",))
        spec = PartitionSpec("core")
        self.fn = jax.jit(
            shard_map(_body, mesh=self.mesh,
                      in_specs=(spec,) * (len(in_names) + len(out_names)),
                      out_specs=(spec,) * len(out_names), check_rep=False),
            keep_unused=True)

    def stage(self, in_maps):
        import jax
        from jax.sharding import PartitionSpec
        concat = [
            np.concatenate([np.asarray(in_maps[c][n]) for c in range(self.n_cores)], axis=0)
            for n in self.in_names
        ] + [np.concatenate([z] * self.n_cores, axis=0) for z in self.zero_outs]
        sharding = jax.sharding.NamedSharding(self.mesh, PartitionSpec("core"))
        return [jax.device_put(a, sharding) for a in concat]

    def run(self, staged):
        outs = self.fn(*staged)
        self.jax.block_until_ready(outs)
        return outs

    def run_to_maps(self, staged):
        outs = self.run(staged)
        res = []
        for c in range(self.n_cores):
            m = {}
            for i, n in enumerate(self.out_names):
                g = np.asarray(outs[i])
                per = g.shape[0] // self.n_cores
                m[n] = g[c * per:(c + 1) * per]
            res.append(m)
        return res


def get_runner(reps: int = 1, phases: int = 4, variant: str = "full"):
    key = (reps, phases, variant)
    if key not in _RUNNER_CACHE:
        nc = build_nc(reps, phases, variant)
        _RUNNER_CACHE[key] = _Runner(nc, N_CORES)
    return _RUNNER_CACHE[key]


def make_in_maps(x, Wq, bq, Wk, bk, Wv, bv, Wp, bp):
    import ml_dtypes
    bf = ml_dtypes.bfloat16
    x = np.asarray(x, dtype=np.float32)
    weights = {
        "Wq": np.asarray(Wq, bf), "Wk": np.asarray(Wk, bf),
        "Wv": np.asarray(Wv, bf), "Wp": np.asarray(Wp, bf),
    }
    bqT = np.ascontiguousarray(np.asarray(bq, np.float32).reshape(KS, P).T)
    bkT = np.ascontiguousarray(np.asarray(bk, np.float32).reshape(KS, P).T)
    bvB = np.ascontiguousarray(np.broadcast_to(np.asarray(bv, np.float32), (P, C)))
    bpB = np.ascontiguousarray(np.broadcast_to(np.asarray(bp, np.float32), (P, C)))
    in_maps = []
    for b in range(B):
        in_maps.append({
            "xT": np.ascontiguousarray(x[b].T.astype(bf)),
            "Wq": weights["Wq"], "Wk": weights["Wk"],
            "Wv": weights["Wv"], "Wp": weights["Wp"],
            "bqT": bqT, "bkT": bkT, "bvB": bvB, "bpB": bpB,
        })
    return in_maps


def kernel(x, Wq, bq, Wk, bk, Wv, bv, Wp, bp):
    runner = get_runner(reps=1)
    in_maps = make_in_maps(x, Wq, bq, Wk, bk, Wv, bv, Wp, bp)
    staged = runner.stage(in_maps)
    res = runner.run_to_maps(staged)
    return np.stack([np.asarray(res[b]["y"], np.float32) for b in range(B)],
                    axis=0)
